# revision 1
# baseline (speedup 1.0000x reference)
"""Multi-head attention (B=8, S=1024, D=768, H=12, DH=64) on 8 TRN2 NeuronCores.

Strategy: pure data parallelism over batch — core b computes batch element b
end-to-end (no collectives). Per core, activations are kept in transposed
[feature, token] layout so every matmul contracts over the partition dim:

  xT [768,1024]  (host-transposed, bf16)
  qT/kT/vT per head-pair [128,1024] = Wqkv_pair.T @ xT   (PE, k=128, m=128)
  S^T per (pair, tchunk, shalf) [128,1024] = (even|odd) scores, k=64 row-tiled
  P = exp(S^T * 0.125)        (ACT, psum->sbuf bf16)
  O^T[65,512] += [V|1].T @ P  (PE; row 64 = softmax denominator for free)
  normalize: recip(denom) -> DRAM -> partition-broadcast DMA -> DVE multiply
  Y [1024,768] = OT.T @ Wo + bo  (PE k=128; DVE bias-add; natural layout out)

All matmul inputs bf16 (fp32 PSUM accumulation); output f32.
"""

import sys

sys.path.insert(0, "/opt/trn_rl_repo")

import numpy as np
import ml_dtypes

B, S, D = 8, 1024, 768
H = 12
DH = 64
NPAIR = 6  # head pairs
NDC = 6  # 128-wide chunks of D
NTC = 8  # 128-wide chunks of S (key/t side)
NSC = 8  # 128-wide chunks of S (query/s side)

_BF16 = ml_dtypes.bfloat16

_cache = {}


def _build_program():
    import concourse.bass as bass
    import concourse.bacc as bacc
    import concourse.tile as tile
    from concourse import mybir

    F32 = mybir.dt.float32
    BF16 = mybir.dt.bfloat16
    Exp = mybir.ActivationFunctionType.Exp

    nc = bacc.Bacc("TRN2", target_bir_lowering=False, debug=False)

    # ---- DRAM I/O (per core) ----
    xT_d = nc.dram_tensor("xT", [D, S], BF16, kind="ExternalInput")
    wqkv_d = nc.dram_tensor("wqkv", [NPAIR, 128, 3 * D], BF16, kind="ExternalInput")
    wo_d = nc.dram_tensor("wo", [128, NDC * D], BF16, kind="ExternalInput")
    bqkv_d = nc.dram_tensor("bqkv", [128, 18], F32, kind="ExternalInput")
    bo_d = nc.dram_tensor("bo", [1, D], F32, kind="ExternalInput")
    ident_d = nc.dram_tensor("ident", [128, 64], BF16, kind="ExternalInput")
    y_d = nc.dram_tensor("y", [S, D], F32, kind="ExternalOutput")

    denom_d = nc.dram_tensor("denom_scr", [H, S], F32, kind="Internal")

    with tile.TileContext(nc) as tc:
        import contextlib

        ctx = contextlib.ExitStack()
        with ctx:
            const = ctx.enter_context(tc.tile_pool(name="const", bufs=1))
            wpool = ctx.enter_context(tc.tile_pool(name="wpool", bufs=1))
            qkv = ctx.enter_context(tc.tile_pool(name="qkv", bufs=3))
            vn_pool = ctx.enter_context(tc.tile_pool(name="vn", bufs=4))
            ot_pool = ctx.enter_context(tc.tile_pool(name="ot", bufs=1))
            e_pool = ctx.enter_context(tc.tile_pool(name="e", bufs=6))
            r_pool = ctx.enter_context(tc.tile_pool(name="r", bufs=4))
            y_pool = ctx.enter_context(tc.tile_pool(name="ysb", bufs=3))
            ps = ctx.enter_context(tc.tile_pool(name="ps", bufs=1, space="PSUM"))

            # ---- inputs to SBUF; critical-path first (xT + pair-0 weights) ----
            # xT resident tile; first projection's weights go first, then xT
            # chunks (fine-grained deps), then the rest.
            xt_all = wpool.tile([128, NDC * S], BF16, name="xt_all")
            xT = [xt_all[:, dc * S : (dc + 1) * S] for dc in range(NDC)]
            xt_src = xT_d.rearrange("(dc p) s -> p dc s", p=128)

            w_sb = {}
            wqkv_t = {}
            for p in range(NPAIR):
                wqkv_t[p] = wpool.tile([128, 3 * D], BF16, name=f"wqkv{p}")
                for i, wname in enumerate(("q", "k", "v")):
                    w_sb[wname, p] = wqkv_t[p][:, i * D : (i + 1) * D]

            nc.sync.dma_start(w_sb["q", 0], wqkv_d[0, :, 0:D])
            nc.sync.dma_start(xT[0], xt_src[:, 0, :])
            nc.sync.dma_start(xT[1], xt_src[:, 1, :])
            nc.sync.dma_start(w_sb["k", 0], wqkv_d[0, :, D : 2 * D])
            nc.sync.dma_start(xT[2], xt_src[:, 2, :])
            nc.sync.dma_start(xT[3], xt_src[:, 3, :])
            nc.sync.dma_start(w_sb["v", 0], wqkv_d[0, :, 2 * D : 3 * D])
            nc.sync.dma_start(xT[4], xt_src[:, 4, :])
            nc.sync.dma_start(xT[5], xt_src[:, 5, :])
            bqkv = const.tile([128, 18], F32)
            nc.sync.dma_start(bqkv, bqkv_d[:, :])
            ident = const.tile([128, 64], BF16)
            nc.sync.dma_start(ident, ident_d[:, :])
            for p in range(1, NPAIR):
                nc.sync.dma_start(wqkv_t[p], wqkv_d[p, :, :])
            bo_b = const.tile([128, D], F32)
            nc.sync.dma_start(
                bo_b, bass.AP(tensor=bo_d, offset=0, ap=[[0, 128], [1, D]])
            )
            wo_all = wpool.tile([128, NDC * D], BF16, name="wo_all")
            nc.sync.dma_start(wo_all, wo_d[:, :])
            wo_sb = [wo_all[:, dc * D : (dc + 1) * D] for dc in range(NDC)]

            # persistent OT tiles (one per pair, [128, 1024] bf16)
            OT = [ot_pool.tile([128, S], BF16, name=f"OT{p}") for p in range(NPAIR)]

            def projection(p, wname, bias_col):
                """Compute (W_pair.T @ xT + b) -> bf16 [128, 1024] tile."""
                dst = qkv.tile([128, S], BF16, tag=wname, name=f"{wname}T{p}")
                w = w_sb[wname, p]
                for nh in range(2):
                    pst = ps.tile(
                        [128, 512], F32, tag="pp", bufs=2, name=f"pp_{wname}{p}{nh}"
                    )
                    for dc in range(NDC):
                        nc.tensor.matmul(
                            pst,
                            w[:, dc * 128 : (dc + 1) * 128],
                            xT[dc][:, nh * 512 : (nh + 1) * 512],
                            start=(dc == 0),
                            stop=(dc == NDC - 1),
                        )
                    # High priority: these evacs gate the next pair's scores;
                    # they must jump the DVE queue ahead of normalize work.
                    with tc.high_priority(offset=300):
                        nc.vector.tensor_scalar_add(
                            dst[:, nh * 512 : (nh + 1) * 512],
                            pst,
                            bqkv[:, bias_col : bias_col + 1],
                        )
                return dst

            def proj_and_vn(p):
                """Projections + V-natural transpose for pair p."""
                qT = projection(p, "q", 0 * 6 + p)
                kT = projection(p, "k", 1 * 6 + p)
                vT = projection(p, "v", 2 * 6 + p)
                vnat = []
                for h2 in range(2):
                    vps = ps.tile([128, 512], BF16, tag="pp", bufs=2, name=f"vn{p}_{h2}")
                    for tcb in range(NTC):
                        nc.tensor.transpose(
                            vps[:, tcb * 64 : (tcb + 1) * 64],
                            vT[h2 * 64 : (h2 + 1) * 64, tcb * 128 : (tcb + 1) * 128],
                            ident[h2 * 64 : (h2 + 1) * 64, :],
                        )
                    vn = vn_pool.tile([128, NTC * 65], BF16, name=f"vnat{p}_{h2}")
                    vn_r = vn.rearrange("a (b c) -> a b c", c=65)
                    nc.vector.tensor_copy(
                        vn_r[:, :, 0:64], vps.rearrange("a (b c) -> a b c", c=64)
                    )
                    nc.vector.memset(vn_r[:, :, 64:65], 1.0)
                    vnat.append(vn)
                return qT, kT, vnat

            def normalize(p, h2, ou_t, ssl, sh):
                """recip(denom) -> DRAM roundtrip -> partition-bcast -> mult."""
                h = 2 * p + h2
                n = ssl.stop - ssl.start
                rt = r_pool.tile([65, n], F32, tag="rt", bufs=3, name="rt")
                nc.vector.reciprocal(out=rt[64:65, :], in_=ou_t[64:65, ssl])
                nc.sync.dma_start(denom_d[h, ssl], rt[64:65, :])
                rb = r_pool.tile([64, n], F32, tag="rb", bufs=3, name="rb")
                nc.sync.dma_start(
                    rb,
                    bass.AP(
                        tensor=denom_d,
                        offset=h * S + ssl.start,
                        ap=[[0, 64], [1, n]],
                    ),
                )
                nc.vector.tensor_mul(
                    OT[p][h2 * 64 : (h2 + 1) * 64, ssl], ou_t[0:64, ssl], rb
                )

            for p in range(NPAIR):
                qT, kT, vnat = proj_and_vn(p)

                # attention core; s split in halves to fit PSUM
                ou = [
                    r_pool.tile([65, S], F32, tag="ou", bufs=4, name=f"ou{p}_{h2}")
                    for h2 in range(2)
                ]
                for sh in range(2):
                    ssl = slice(sh * 512, (sh + 1) * 512)
                    Ops = [
                        ps.tile([65, 512], F32, tag="o", bufs=2, name=f"o{p}_{sh}_{h2}")
                        for h2 in range(2)
                    ]
                    for tcb in range(NTC):
                        st = ps.tile([128, S], F32, tag="s", bufs=2, name=f"s{p}_{sh}_{tcb}")
                        nc.tensor.matmul(
                            st[:, 0:512],
                            kT[0:64, tcb * 128 : (tcb + 1) * 128],
                            qT[0:64, ssl],
                            start=True,
                            stop=True,
                        )
                        nc.tensor.matmul(
                            st[:, 512:1024],
                            kT[64:128, tcb * 128 : (tcb + 1) * 128],
                            qT[64:128, ssl],
                            start=True,
                            stop=True,
                        )
                        et = e_pool.tile([128, S], BF16, name="expS")
                        nc.scalar.activation(et, st, Exp, scale=0.125)
                        for h2 in range(2):
                            nc.tensor.matmul(
                                Ops[h2][:, :],
                                vnat[h2][:, tcb * 65 : (tcb + 1) * 65],
                                et[:, h2 * 512 : (h2 + 1) * 512],
                                start=(tcb == 0),
                                stop=(tcb == NTC - 1),
                            )
                    # evacuate unnormalized O (frees psum fast)
                    for h2 in range(2):
                        nc.vector.tensor_copy(ou[h2][:, ssl], Ops[h2])
                    if p == NPAIR - 1:
                        # last pair: normalize each s-half as soon as it's
                        # done, so the output projection isn't tail-blocked.
                        for h2 in range(2):
                            normalize(p, h2, ou[h2], ssl, sh)
                if p < NPAIR - 1:
                    # normalize per head, full width, off the critical path
                    for h2 in range(2):
                        normalize(p, h2, ou[h2], slice(0, S), None)

            # ---- output projection: Y[sc] = sum_dc OT[dc][:, sc].T @ Wo[dc] + bo
            for sc in range(NSC):
                yps = ps.tile([128, D], F32, tag="s", bufs=2, name=f"y{sc}")
                for dc in range(NDC):
                    lhsT = OT[dc][:, sc * 128 : (sc + 1) * 128]
                    nc.tensor.matmul(
                        yps[:, 0:512],
                        lhsT,
                        wo_sb[dc][:, 0:512],
                        start=(dc == 0),
                        stop=(dc == NDC - 1),
                    )
                    nc.tensor.matmul(
                        yps[:, 512:768],
                        lhsT,
                        wo_sb[dc][:, 512:768],
                        start=(dc == 0),
                        stop=(dc == NDC - 1),
                    )
                yt = y_pool.tile([128, D], F32, name="yt")
                nc.vector.tensor_add(yt, yps, bo_b)
                nc.sync.dma_start(y_d[sc * 128 : (sc + 1) * 128, :], yt)

    nc.compile()
    return nc


def _prep_inputs(x, Wq, bq, Wk, bk, Wv, bv, Wo, bo):
    """Host-side layout transforms + bf16 casts."""
    x = np.asarray(x)
    # xT per batch: [B, D, S] bf16
    xT = np.ascontiguousarray(x.transpose(0, 2, 1)).astype(_BF16)

    def pack_w(W):
        # W [H, D, DH] -> [NPAIR, 128(d_sub), D(dc*128+m)] where m in 0..127
        # indexes (head-in-pair, e): value[p, d_sub, dc*128+m] = W[2p + m//64, dc*128+d_sub, m%64]
        Wp = np.empty((NPAIR, 128, D), np.float32)
        W = np.asarray(W, np.float32)
        for p in range(NPAIR):
            blk = np.concatenate([W[2 * p], W[2 * p + 1]], axis=1)  # [D, 128]
            # want [d_sub, dc*128+m] = blk[dc*128+d_sub, m]
            Wp[p] = blk.reshape(NDC, 128, 128).transpose(1, 0, 2).reshape(128, D)
        return Wp

    # q|k|v blocks side by side: [NPAIR, 128, 3*768]
    wqkv = np.concatenate([pack_w(Wq), pack_w(Wk), pack_w(Wv)], axis=2).astype(_BF16)

    bqkv = np.empty((128, 18), np.float32)
    for j, b_ in enumerate((bq, bk, bv)):
        b_ = np.asarray(b_, np.float32)
        for p in range(NPAIR):
            bqkv[:, j * 6 + p] = np.concatenate([b_[2 * p], b_[2 * p + 1]])

    Wo = np.asarray(Wo, np.float32)
    # [128(d_sub), NDC*768]: wo[:, dc*768 + j] = Wo[dc*128 + d_sub, j]
    wo = Wo.reshape(NDC, 128, D).transpose(1, 0, 2).reshape(128, NDC * D).astype(_BF16)

    bo_h = np.asarray(bo, np.float32).reshape(1, D)

    ident = np.zeros((128, 64), np.float32)
    ident[0:64] = np.eye(64)
    ident[64:128] = np.eye(64)
    ident = ident.astype(_BF16)

    shared = {
        "wqkv": wqkv,
        "wo": wo,
        "bqkv": bqkv,
        "bo": bo_h,
        "ident": ident,
    }
    return xT, shared


def kernel(x, Wq, bq, Wk, bk, Wv, bv, Wo, bo):
    from concourse.bass_utils import run_bass_kernel_spmd

    if "nc" not in _cache:
        _cache["nc"] = _build_program()
    nc = _cache["nc"]

    xT, shared = _prep_inputs(x, Wq, bq, Wk, bk, Wv, bv, Wo, bo)
    in_maps = [dict(shared, xT=np.ascontiguousarray(xT[b])) for b in range(B)]
    res = run_bass_kernel_spmd(nc, in_maps, core_ids=list(range(B)))
    y = np.stack([res.results[b]["y"] for b in range(B)], axis=0)
    return y.astype(np.float32)



# revision 3
# speedup vs baseline: 1.0057x; 1.0057x over previous
"""Multi-head attention (B=8, S=1024, D=768, H=12, DH=64) on 8 TRN2 NeuronCores.

Strategy: pure data parallelism over batch — core b computes batch element b
end-to-end (no collectives). Optimized v2:

  - q/k projections (bf16, PE): qT/kT [128(2 heads' e), 1024] per pair.
    q evacuated straight to fp8 hi/lo pair (DVE), k straight to fp8 (Pool).
    bk dropped entirely (softmax-invariant).
  - v projection operand-swapped (bf16, PE): lhsT = xT chunks (stationary),
    rhs = Wv natural (moving) -> V [t, e] natural tiles directly, with a
    ones-column per head appended for free denominators.
  - scores: fp8e4m3 DoubleRow matmuls (0.5 cyc/row): lhsT = k8 broadcast
    (two identical k-tiles, stride 0), rhs = (q_hi, q_lo) -> computes
    k.T @ (q_hi + q_lo): q at ~bf16 precision, k plain fp8. rel err ~5e-3.
  - exp on ACT -> P^T bf16 tiles (the only ACT work, ~100us = the wall).
  - AV operand-swapped: P^T chunks stationary, [V | 1] moving: cost counts
    only moving columns -> 2x cheaper than the m=65 orientation, and the
    denominator lands as a per-partition column -> normalize is a cheap
    per-partition tensor_scalar divide (Pool), no DRAM broadcast roundtrip.
  - O^T via DMA transpose (SBUF->SBUF, no PE/PSUM cost).
  - out-proj split: pairs 0-4 accumulated into a bf16 partial during the
    last pair's exp window; pair 5 folded in per s-chunk at the tail.
  - emission is software-pipelined at ~0.5us granularity: after each score
    tile (2 matmuls + exp) the emitter appends independent PE work (next
    pair's projections, v-proj, AV of pair p-2, out-proj partials) so the
    in-order PE queue never blocks on the scores/exp PSUM ping-pong.

All matmul inputs bf16/fp8 (fp32 PSUM accumulation); output f32.
"""

import sys

sys.path.insert(0, "/opt/trn_rl_repo")

import numpy as np
import ml_dtypes

B, S, D = 8, 1024, 768
H = 12
DH = 64
NPAIR = 6  # head pairs
NDC = 6  # 128-wide chunks of D
NTC = 8  # 128-wide chunks of S (key/t side)
NSC = 8  # 128-wide chunks of S (query/s side)

_BF16 = ml_dtypes.bfloat16

_cache = {}


def _build_program():
    import concourse.bass as bass
    import concourse.bacc as bacc
    import concourse.tile as tile
    from concourse import mybir

    F32 = mybir.dt.float32
    BF16 = mybir.dt.bfloat16
    FP8 = mybir.dt.float8e4
    Exp = mybir.ActivationFunctionType.Exp
    Alu = mybir.AluOpType
    DR = mybir.MatmulPerfMode.DoubleRow

    nc = bacc.Bacc("TRN2", target_bir_lowering=False, debug=False)

    # ---- DRAM I/O (per core) ----
    xT_d = nc.dram_tensor("xT", [D, S], BF16, kind="ExternalInput")
    wqk_d = nc.dram_tensor("wqk", [NPAIR, 128, 2 * D], BF16, kind="ExternalInput")
    wv_d = nc.dram_tensor("wv", [NDC, 128, D], BF16, kind="ExternalInput")
    wo_d = nc.dram_tensor("wo", [NDC, 128, D], BF16, kind="ExternalInput")
    bq_d = nc.dram_tensor("bq", [128, NPAIR], F32, kind="ExternalInput")
    bv_d = nc.dram_tensor("bv", [1, D], F32, kind="ExternalInput")
    bo_d = nc.dram_tensor("bo", [1, D], F32, kind="ExternalInput")
    ident_d = nc.dram_tensor("ident", [128, 128], BF16, kind="ExternalInput")
    y_d = nc.dram_tensor("y", [S, D], F32, kind="ExternalOutput")

    with tile.TileContext(nc) as tc:
        import contextlib

        ctx = contextlib.ExitStack()
        with ctx:
            const = ctx.enter_context(tc.tile_pool(name="const", bufs=1))
            wpool = ctx.enter_context(tc.tile_pool(name="wpool", bufs=1))
            qk8 = ctx.enter_context(tc.tile_pool(name="qk8", bufs=3))
            vpool = ctx.enter_context(tc.tile_pool(name="vp", bufs=1))
            pt_pool = ctx.enter_context(tc.tile_pool(name="pt", bufs=45))
            on_pool = ctx.enter_context(tc.tile_pool(name="on", bufs=12))
            ov_sb_pool = ctx.enter_context(tc.tile_pool(name="ovsb", bufs=4))
            ot_pool = ctx.enter_context(tc.tile_pool(name="ot", bufs=1))
            yp_pool = ctx.enter_context(tc.tile_pool(name="ypart", bufs=1))
            y_pool = ctx.enter_context(tc.tile_pool(name="ysb", bufs=4))
            ps = ctx.enter_context(tc.tile_pool(name="ps", bufs=1, space="PSUM"))

            # ---- input DMAs; critical path first (pair-0 qk weights + x) ----
            xt_all = wpool.tile([128, NDC * S], BF16, name="xt_all")
            xT = [xt_all[:, dc * S : (dc + 1) * S] for dc in range(NDC)]
            xt_src = xT_d.rearrange("(dc p) s -> p dc s", p=128)

            wqk_t = {}
            for p in range(NPAIR):
                wqk_t[p] = wpool.tile([128, 2 * D], BF16, name=f"wqk{p}")
            wq_sb = {p: wqk_t[p][:, 0:D] for p in range(NPAIR)}
            wk_sb = {p: wqk_t[p][:, D : 2 * D] for p in range(NPAIR)}

            # two hwdge queues (SP + ACT): critical-path tensors first
            bq_sb = const.tile([128, NPAIR], F32)
            wv_all = wpool.tile([128, NDC * D], BF16, name="wv_all")
            wv_sb = [wv_all[:, dc * D : (dc + 1) * D] for dc in range(NDC)]
            bv_b = const.tile([128, D], F32)
            bo_b = const.tile([128, D], F32)
            ident = const.tile([128, 128], BF16)
            wo_all = wpool.tile([128, NDC * D], BF16, name="wo_all")
            wo_sb = [wo_all[:, dc * D : (dc + 1) * D] for dc in range(NDC)]

            nc.sync.dma_start(wqk_t[0][:, 0:D], wqk_d[0, :, 0:D])
            nc.scalar.dma_start(bq_sb, bq_d[:, :])
            nc.scalar.dma_start(wqk_t[0][:, D : 2 * D], wqk_d[0, :, D : 2 * D])
            for dc in range(NDC):
                eng = (nc.sync, nc.scalar)[dc % 2]
                eng.dma_start(xT[dc], xt_src[:, dc, :])
            for p in range(1, NPAIR):
                nc.sync.dma_start(wqk_t[p], wqk_d[p, :, :])
            for dc in range(NDC):
                nc.sync.dma_start(wv_sb[dc], wv_d[dc, :, :])
            nc.sync.dma_start(
                bv_b, bass.AP(tensor=bv_d, offset=0, ap=[[0, 128], [1, D]])
            )
            nc.sync.dma_start(
                bo_b, bass.AP(tensor=bo_d, offset=0, ap=[[0, 128], [1, D]])
            )
            nc.sync.dma_start(ident, ident_d[:, :])
            for dc in range(NDC):
                nc.sync.dma_start(wo_sb[dc], wo_d[dc, :, :])

            # persistent V tiles: [t 128, 12*65] (col h*65+64 is the ones col)
            VW = H * 65  # 780
            v_t = [vpool.tile([128, VW], BF16, name=f"V{t}") for t in range(NTC)]
            # persistent OT tiles per pair: [of 128, 1024 s]
            OT = [ot_pool.tile([128, S], BF16, name=f"OT{p}") for p in range(NPAIR)]
            # partial y (pairs 0..4 + bias), per s-chunk, bf16
            ypart = yp_pool.tile([128, NSC * D], BF16, name="ypart")

            warm = const.tile([128, 512], BF16, name="warm")
            nc.vector.memset(warm, 0.0)
            for i in range(5):
                wps = ps.tile([128, 512], F32, tag="pp", bufs=2, name=f"warm{i}")
                nc.tensor.matmul(
                    wps[:, 0:256], warm[:, 0:128], warm[:, 0:256],
                    start=True, stop=True,
                )

            q8 = {}
            k8 = {}
            pts = {}  # pts[p][h2][tcb] -> P^T tile

            # ---------- chunk emitters (each ~0.4-1.6us of PE work) ----------

            proj_pst = {}

            def proj_chunk(p, which, nh, part=None):
                """One 512-col half of a q/k projection + its evacuation.

                part=0: first 3 d-chunks; part=1: last 3 + evac; None: all.
                """
                w = wq_sb[p] if which == "q" else wk_sb[p]
                if part == 1:
                    pst = proj_pst.pop((p, which, nh))
                else:
                    pst = ps.tile(
                        [128, 512], F32, tag="pp", bufs=2, name=f"pj{p}{which}{nh}"
                    )
                rng = {None: range(NDC), 0: range(3), 1: range(3, NDC)}[part]
                for dc in rng:
                    nc.tensor.matmul(
                        pst,
                        w[:, dc * 128 : (dc + 1) * 128],
                        xT[dc][:, nh * 512 : (nh + 1) * 512],
                        start=(dc == 0),
                        stop=(dc == NDC - 1),
                    )
                if part == 0:
                    proj_pst[(p, which, nh)] = pst
                    return
                sl = slice(nh * 512, (nh + 1) * 512)
                with tc.high_priority(offset=200):
                    if which == "q":
                        hi = q8[p][:, 0:S]
                        lo = q8[p][:, S : 2 * S]
                        nc.vector.tensor_scalar_add(
                            hi[:, sl], pst, bq_sb[:, p : p + 1]
                        )
                        nc.vector.scalar_tensor_tensor(
                            lo[:, sl], pst, bq_sb[:, p : p + 1], hi[:, sl],
                            Alu.add, Alu.subtract,
                        )
                    else:
                        nc.vector.tensor_copy(k8[p][:, sl], pst)

            def score_chunk(p, h2, tcb, split_exp=False):
                """One scores tile (2 fp8-DR matmuls) + exp -> P^T tile.

                split_exp: exp per s-half right behind its matmul — used for
                the first tiles so ACT starts before the q-nh1 evac lands.
                """
                q3 = q8[p].rearrange("p (two s) -> p two s", two=2)
                psl = slice(h2 * 64, (h2 + 1) * 64)
                st = ps.tile([128, S], F32, tag="sc", bufs=2, name=f"s{p}{h2}{tcb}")
                lhsT = (
                    k8[p][psl, tcb * 128 : (tcb + 1) * 128]
                    .unsqueeze(1)
                    .broadcast_to((64, 2, 128))
                )
                pt = pt_pool.tile([128, S], BF16, tag="pt", name=f"pt{p}{h2}{tcb}")
                for sh in range(2):
                    ssl = slice(sh * 512, (sh + 1) * 512)
                    nc.tensor.matmul(
                        st[:, ssl],
                        lhsT,
                        q3[psl, :, ssl],
                        start=True,
                        stop=True,
                        perf_mode=DR,
                    )
                    if split_exp:
                        nc.scalar.activation(pt[:, ssl], st[:, ssl], Exp, scale=0.125)
                if not split_exp:
                    nc.scalar.activation(pt, st, Exp, scale=0.125)
                pts[p][h2][tcb] = pt

            def v_chunk(tcb, half):
                """v-proj for one (t-chunk, col-half); swapped operands."""
                c0, c1 = ((0, 512), (512, 768))[half]
                n = c1 - c0
                pv = ps.tile([128, 512], F32, tag="pp", bufs=2, name=f"vv{tcb}{half}")
                for dc in range(NDC):
                    nc.tensor.matmul(
                        pv[:, 0:n],
                        xT[dc][:, tcb * 128 : (tcb + 1) * 128],
                        wv_sb[dc][:, c0:c1],
                        start=(dc == 0),
                        stop=(dc == NDC - 1),
                    )
                nh = n // 64
                h0 = c0 // 64
                dst = v_t[tcb][:, h0 * 65 : h0 * 65 + nh * 65]
                dst3 = dst.rearrange("p (h e) -> p h e", e=65)[:, :, 0:64]
                src3 = pv[:, 0:n].rearrange("p (h e) -> p h e", e=64)
                bias3 = bv_b[:, c0:c1].rearrange("p (h e) -> p h e", e=64)
                nc.vector.tensor_tensor(dst3, src3, bias3, Alu.add)
                if half == 1:
                    ones3 = v_t[tcb].rearrange("p (h e) -> p h e", e=65)[:, :, 64:65]
                    nc.vector.memset(ones3, 1.0)

            onrm_t = {}

            av_ov = {}

            def av_chunk(p, sc, heads=(0, 1), transpose="dma", quarter=None):
                """O for one (pair, s-chunk): AV matmuls, normalize, transpose.

                transpose: "dma" (SP hwdge queue), "pe" (tail: PE + Pool copy),
                or None (first-head half of a split pair).
                """
                if (p, sc) in onrm_t:
                    onrm = onrm_t.pop((p, sc))
                else:
                    onrm = on_pool.tile(
                        [128, 128], BF16, tag="on", name=f"on{p}{sc}"
                    )
                if quarter is not None:
                    heads = (quarter // 2,)
                    tcbs = range(4) if quarter % 2 == 0 else range(4, NTC)
                else:
                    tcbs = range(NTC)
                for h2 in heads:
                    h = 2 * p + h2
                    if quarter is not None and quarter % 2 == 1:
                        ov = av_ov.pop((p, sc, h2))
                    else:
                        ov = ps.tile(
                            [128, 65], F32, tag="ov", bufs=2, name=f"ov{p}{sc}{h2}"
                        )
                    for tcb in tcbs:
                        nc.tensor.matmul(
                            ov,
                            pts[p][h2][tcb][:, sc * 128 : (sc + 1) * 128],
                            v_t[tcb][:, h * 65 : h * 65 + 65],
                            start=(tcb == 0),
                            stop=(tcb == NTC - 1),
                        )
                    if quarter is not None and quarter % 2 == 0:
                        av_ov[(p, sc, h2)] = ov
                        onrm_t[(p, sc)] = onrm
                        return
                    rsb = ov_sb_pool.tile(
                        [128, 1], F32, tag="ovsb", name=f"r{p}{sc}{h2}"
                    )
                    nc.vector.reciprocal(out=rsb, in_=ov[:, 64:65])
                    nc.vector.tensor_scalar_mul(
                        onrm[:, h2 * 64 : (h2 + 1) * 64], ov[:, 0:64], rsb
                    )
                if quarter is not None and quarter < 3:
                    onrm_t[(p, sc)] = onrm
                    return
                if transpose is None:
                    onrm_t[(p, sc)] = onrm
                elif transpose == "dma":
                    nc.sync.dma_start_transpose(
                        OT[p][:, sc * 128 : (sc + 1) * 128], onrm
                    )
                else:
                    tp = ps.tile([128, 128], BF16, tag="ov", bufs=2, name=f"tp{sc}")
                    nc.tensor.transpose(tp, onrm, ident)
                    nc.vector.tensor_copy(OT[p][:, sc * 128 : (sc + 1) * 128], tp)

            def ypart_chunk(sc, half=None):
                """Out-proj partial: pairs 0..4 + bias -> bf16 ypart."""
                cols = ((0, 512), (512, 768))
                if half is not None:
                    cols = (cols[half],)
                for c0, c1 in cols:
                    n = c1 - c0
                    yp = ps.tile([128, 512], F32, tag="pp", bufs=2, name=f"yp{sc}")
                    for p in range(NPAIR - 1):
                        nc.tensor.matmul(
                            yp[:, 0:n],
                            OT[p][:, sc * 128 : (sc + 1) * 128],
                            wo_sb[p][:, c0:c1],
                            start=(p == 0),
                            stop=(p == NPAIR - 2),
                        )
                    nc.vector.tensor_add(
                        ypart[:, sc * D + c0 : sc * D + c1], yp[:, 0:n],
                        bo_b[:, c0:c1],
                    )

            def yfinal_chunk(sc):
                """Fold pair 5 + the bf16 partial (identity matmul), DMA out
                straight from PSUM."""
                yf = ps.tile([128, D], F32, tag="sc", bufs=2, name=f"yf{sc}")
                lhsT = OT[NPAIR - 1][:, sc * 128 : (sc + 1) * 128]
                for c0, c1 in ((0, 512), (512, 768)):
                    nc.tensor.matmul(
                        yf[:, c0:c1], lhsT, wo_sb[NPAIR - 1][:, c0:c1],
                        start=True, stop=False,
                    )
                    nc.tensor.matmul(
                        yf[:, c0:c1], ident,
                        ypart[:, sc * D + c0 : sc * D + c1],
                        start=False, stop=True,
                    )
                ysb = y_pool.tile([128, D], F32, tag="y", name=f"y{sc}")
                if sc % 2 == 0:
                    nc.vector.tensor_copy(ysb, yf)
                else:
                    nc.scalar.copy(ysb, yf)
                eng = (nc.sync, nc.scalar)[sc % 2]
                eng.dma_start(y_d[sc * 128 : (sc + 1) * 128, :], ysb)

            # ---------- interleaved schedule ----------
            # Filler work (est PE ns, thunk) emitted between score chunks so
            # the in-order PE stream never blocks on the scores/exp ping-pong.

            def alloc_pair(p):
                q8[p] = qk8.tile([128, 2 * S], FP8, tag="q8", name=f"q8_{p}")
                k8[p] = qk8.tile([128, S], FP8, tag="k8", name=f"k8_{p}")
                pts[p] = [[None] * NTC for _ in range(2)]

            def proj_fillers(p):
                def mk(which, nh, part):
                    return lambda: proj_chunk(p, which, nh, part)

                out = []
                for which, nh in (("q", 0), ("k", 0), ("q", 1), ("k", 1)):
                    out.append((680, mk(which, nh, 0)))
                    out.append((680, mk(which, nh, 1)))
                return out

            def av_fillers(p):
                return [
                    (480, (lambda sc_: lambda: av_chunk(p, sc_))(sc))
                    for sc in range(NSC)
                ]

            def av_fillers_q(p):
                out = []
                for sc in range(NSC):
                    for q in range(4):
                        out.append(
                            (150,
                             (lambda sc_, q_: lambda: av_chunk(p, sc_, quarter=q_))(sc, q))
                        )
                return out

            v_fillers = [
                (1350 if half == 0 else 700,
                 (lambda t_, h_: lambda: v_chunk(t_, h_))(tcb, half))
                for tcb in range(NTC)
                for half in range(2)
            ]

            def av5_h0_fillers():
                out = []
                for sc in range(NSC):
                    for q in (0, 1):
                        out.append(
                            (130,
                             (lambda sc_, q_: lambda: av_chunk(
                                 5, sc_, quarter=q_, transpose=None))(sc, q))
                        )
                return out

            ypart_fillers = [
                (1100 if h == 0 else 580,
                 (lambda sc_, h_: lambda: ypart_chunk(sc_, h_))(sc, h))
                for sc in range(NSC)
                for h in range(2)
            ]

            fillers_by_pair = {
                0: proj_fillers(1),
                1: v_fillers[:8] + proj_fillers(2),
                2: v_fillers[8:] + proj_fillers(3) + av_fillers(0),
                3: proj_fillers(4) + av_fillers(1),
                4: proj_fillers(5) + av_fillers(2) + av_fillers_q(3),
                5: av_fillers_q(4) + av5_h0_fillers() + ypart_fillers,
            }

            alloc_pair(0)
            proj_chunk(0, "q", 0)
            proj_chunk(0, "k", 0)
            # first two score tiles, s-half 0 only (q-nh1 not needed yet):
            # emitted as split matmuls so ACT starts ~2us earlier.
            intro_st = {}
            q3_0 = q8[0].rearrange("p (two s) -> p two s", two=2)
            for tcb in range(2):
                st = ps.tile([128, S], F32, tag="sc", bufs=2, name=f"i{tcb}")
                pt = pt_pool.tile([128, S], BF16, tag="pt", name=f"ipt{tcb}")
                lhsT = (
                    k8[0][0:64, tcb * 128 : (tcb + 1) * 128]
                    .unsqueeze(1)
                    .broadcast_to((64, 2, 128))
                )
                nc.tensor.matmul(
                    st[:, 0:512], lhsT, q3_0[0:64, :, 0:512],
                    start=True, stop=True, perf_mode=DR,
                )
                nc.scalar.activation(pt[:, 0:512], st[:, 0:512], Exp, scale=0.125)
                intro_st[tcb] = (st, pt, lhsT)
            proj_chunk(0, "q", 1)
            for tcb in range(2):
                st, pt, lhsT = intro_st[tcb]
                nc.tensor.matmul(
                    st[:, 512:S], lhsT, q3_0[0:64, :, 512:S],
                    start=True, stop=True, perf_mode=DR,
                )
                nc.scalar.activation(pt[:, 512:S], st[:, 512:S], Exp, scale=0.125)
                pts[0][0][tcb] = pt
            fillers_by_pair[0] = [(680, lambda: proj_chunk(0, "k", 1))] + \
                fillers_by_pair[0]

            FILL_NS = 580  # target filler PE-ns per score chunk
            for p in range(NPAIR):
                if p + 1 < NPAIR:
                    alloc_pair(p + 1)
                fill = list(fillers_by_pair[p])
                fi = 0
                for h2 in range(2):
                    for tcb in range(NTC):
                        if p == 0 and h2 == 0 and tcb < 2:
                            continue  # emitted in the intro above
                        score_chunk(p, h2, tcb)
                        budget = FILL_NS
                        while fi < len(fill) and budget > 0:
                            est, thunk = fill[fi]
                            thunk()
                            budget -= est
                            fi += 1
                while fi < len(fill):
                    fill[fi][1]()
                    fi += 1
                if p >= 2:
                    del pts[p - 2]

            # tail: pair-5 head-11 AV per s-chunk (head 10 was a filler),
            # PE-transpose + Pool copy (no DMA queue latency), then the final
            # out-proj chunk for that s-chunk.
            for sc in range(NSC):
                av_chunk(5, sc, heads=(1,), transpose="pe")
                yfinal_chunk(sc)

    nc.compile()
    return nc


def _prep_inputs(x, Wq, bq, Wk, bk, Wv, bv, Wo, bo):
    """Host-side layout transforms + bf16 casts."""
    x = np.asarray(x)
    xT = np.ascontiguousarray(x.transpose(0, 2, 1)).astype(_BF16)

    def pack_w(W):
        # W [H, D, DH] -> [NPAIR, 128(d_sub), D] where col dc*128+m holds
        # W[2p + m//64, dc*128+d_sub, m%64]
        Wp = np.empty((NPAIR, 128, D), np.float32)
        W = np.asarray(W, np.float32)
        for p in range(NPAIR):
            blk = np.concatenate([W[2 * p], W[2 * p + 1]], axis=1)  # [D, 128]
            Wp[p] = blk.reshape(NDC, 128, 128).transpose(1, 0, 2).reshape(128, D)
        return Wp

    wqk = np.concatenate([pack_w(Wq), pack_w(Wk)], axis=2).astype(_BF16)

    bq_sb = np.empty((128, NPAIR), np.float32)
    bq = np.asarray(bq, np.float32)
    for p in range(NPAIR):
        bq_sb[:, p] = np.concatenate([bq[2 * p], bq[2 * p + 1]])

    # Wv natural [D, H*DH] chunked over d
    Wv = np.asarray(Wv, np.float32)
    wv = Wv.transpose(1, 0, 2).reshape(D, D).reshape(NDC, 128, D).astype(_BF16)
    Wo = np.asarray(Wo, np.float32)
    wo = Wo.reshape(NDC, 128, D).astype(_BF16)

    bv_h = np.asarray(bv, np.float32).reshape(1, D)
    bo_h = np.asarray(bo, np.float32).reshape(1, D)

    shared = {
        "wqk": wqk,
        "wv": wv,
        "wo": wo,
        "bq": bq_sb,
        "bv": bv_h,
        "bo": bo_h,
        "ident": np.eye(128, dtype=np.float32).astype(_BF16),
    }
    return xT, shared


def kernel(x, Wq, bq, Wk, bk, Wv, bv, Wo, bo):
    from concourse.bass_utils import run_bass_kernel_spmd

    if "nc" not in _cache:
        _cache["nc"] = _build_program()
    nc = _cache["nc"]

    xT, shared = _prep_inputs(x, Wq, bq, Wk, bk, Wv, bv, Wo, bo)
    in_maps = [dict(shared, xT=np.ascontiguousarray(xT[b])) for b in range(B)]
    res = run_bass_kernel_spmd(nc, in_maps, core_ids=list(range(B)))
    y = np.stack([res.results[b]["y"] for b in range(B)], axis=0)
    return y.astype(np.float32)


# revision 4
# speedup vs baseline: 1.0219x; 1.0161x over previous
"""Multi-head attention (B=8, S=1024, D=768, H=12, DH=64) on 8 TRN2 NeuronCores.

Strategy: pure data parallelism over batch — core b computes batch element b
end-to-end (no collectives). Optimized v2:

  - q/k projections (bf16, PE): qT/kT [128(2 heads' e), 1024] per pair.
    q evacuated straight to fp8 hi/lo pair (DVE), k straight to fp8 (Pool).
    bk dropped entirely (softmax-invariant).
  - v projection operand-swapped (bf16, PE): lhsT = xT chunks (stationary),
    rhs = Wv natural (moving) -> V [t, e] natural tiles directly, with a
    ones-column per head appended for free denominators.
  - scores: fp8e4m3 DoubleRow matmuls (0.5 cyc/row): lhsT = k8 broadcast
    (two identical k-tiles, stride 0), rhs = (q_hi, q_lo) -> computes
    k.T @ (q_hi + q_lo): q at ~bf16 precision, k plain fp8. rel err ~5e-3.
  - exp on ACT -> P^T bf16 tiles (the only ACT work, ~100us = the wall).
  - AV operand-swapped: P^T chunks stationary, [V | 1] moving: cost counts
    only moving columns -> 2x cheaper than the m=65 orientation, and the
    denominator lands as a per-partition column -> normalize is a cheap
    per-partition tensor_scalar divide (Pool), no DRAM broadcast roundtrip.
  - O^T via DMA transpose (SBUF->SBUF, no PE/PSUM cost).
  - out-proj split: pairs 0-4 accumulated into a bf16 partial during the
    last pair's exp window; pair 5 folded in per s-chunk at the tail.
  - emission is software-pipelined at ~0.5us granularity: after each score
    tile (2 matmuls + exp) the emitter appends independent PE work (next
    pair's projections, v-proj, AV of pair p-2, out-proj partials) so the
    in-order PE queue never blocks on the scores/exp PSUM ping-pong.

All matmul inputs bf16/fp8 (fp32 PSUM accumulation); output f32.
"""

import sys

sys.path.insert(0, "/opt/trn_rl_repo")

import numpy as np
import ml_dtypes

B, S, D = 8, 1024, 768
H = 12
DH = 64
NPAIR = 6  # head pairs
NDC = 6  # 128-wide chunks of D
NTC = 8  # 128-wide chunks of S (key/t side)
NSC = 8  # 128-wide chunks of S (query/s side)

_BF16 = ml_dtypes.bfloat16

_cache = {}


def _build_program():
    import concourse.bass as bass
    import concourse.bacc as bacc
    import concourse.tile as tile
    from concourse import mybir

    F32 = mybir.dt.float32
    BF16 = mybir.dt.bfloat16
    FP8 = mybir.dt.float8e4
    Exp = mybir.ActivationFunctionType.Exp
    Alu = mybir.AluOpType
    DR = mybir.MatmulPerfMode.DoubleRow

    nc = bacc.Bacc("TRN2", target_bir_lowering=False, debug=False)

    # ---- DRAM I/O (per core) ----
    xT_d = nc.dram_tensor("xT", [D, S], BF16, kind="ExternalInput")
    wqk_d = nc.dram_tensor("wqk", [NPAIR, 128, 2 * D], BF16, kind="ExternalInput")
    wv_d = nc.dram_tensor("wv", [NDC, 128, D], BF16, kind="ExternalInput")
    wo_d = nc.dram_tensor("wo", [NDC, 128, D], BF16, kind="ExternalInput")
    bq_d = nc.dram_tensor("bq", [128, NPAIR], F32, kind="ExternalInput")
    bv_d = nc.dram_tensor("bv", [1, D], F32, kind="ExternalInput")
    bo_d = nc.dram_tensor("bo", [1, D], F32, kind="ExternalInput")
    ident_d = nc.dram_tensor("ident", [128, 128], BF16, kind="ExternalInput")
    y_d = nc.dram_tensor("y", [S, D], F32, kind="ExternalOutput")

    with tile.TileContext(nc) as tc:
        import contextlib

        ctx = contextlib.ExitStack()
        with ctx:
            const = ctx.enter_context(tc.tile_pool(name="const", bufs=1))
            wpool = ctx.enter_context(tc.tile_pool(name="wpool", bufs=1))
            qk8 = ctx.enter_context(tc.tile_pool(name="qk8", bufs=3))
            vpool = ctx.enter_context(tc.tile_pool(name="vp", bufs=1))
            pt_pool = ctx.enter_context(tc.tile_pool(name="pt", bufs=45))
            on_pool = ctx.enter_context(tc.tile_pool(name="on", bufs=12))
            ov_sb_pool = ctx.enter_context(tc.tile_pool(name="ovsb", bufs=4))
            ot_pool = ctx.enter_context(tc.tile_pool(name="ot", bufs=1))
            yp_pool = ctx.enter_context(tc.tile_pool(name="ypart", bufs=1))
            y_pool = ctx.enter_context(tc.tile_pool(name="ysb", bufs=4))
            ps = ctx.enter_context(tc.tile_pool(name="ps", bufs=1, space="PSUM"))

            # ---- input DMAs; critical path first (pair-0 qk weights + x) ----
            xt_all = wpool.tile([128, NDC * S], BF16, name="xt_all")
            xT = [xt_all[:, dc * S : (dc + 1) * S] for dc in range(NDC)]
            xt_src = xT_d.rearrange("(dc p) s -> p dc s", p=128)

            wqk_t = {}
            for p in range(NPAIR):
                wqk_t[p] = wpool.tile([128, 2 * D], BF16, name=f"wqk{p}")
            wq_sb = {p: wqk_t[p][:, 0:D] for p in range(NPAIR)}
            wk_sb = {p: wqk_t[p][:, D : 2 * D] for p in range(NPAIR)}

            # two hwdge queues (SP + ACT): critical-path tensors first
            bq_sb = const.tile([128, NPAIR], F32)
            wv_all = wpool.tile([128, NDC * D], BF16, name="wv_all")
            wv_sb = [wv_all[:, dc * D : (dc + 1) * D] for dc in range(NDC)]
            bv_b = const.tile([128, D], F32)
            bo_b = const.tile([128, D], F32)
            ident = const.tile([128, 128], BF16)
            wo_all = wpool.tile([128, NDC * D], BF16, name="wo_all")
            wo_sb = [wo_all[:, dc * D : (dc + 1) * D] for dc in range(NDC)]

            nc.sync.dma_start(wqk_t[0][:, 0:D], wqk_d[0, :, 0:D])
            nc.scalar.dma_start(bq_sb, bq_d[:, :])
            nc.scalar.dma_start(wqk_t[0][:, D : 2 * D], wqk_d[0, :, D : 2 * D])
            for dc in range(NDC):
                eng = (nc.sync, nc.scalar)[dc % 2]
                eng.dma_start(xT[dc], xt_src[:, dc, :])
            for p in range(1, NPAIR):
                nc.sync.dma_start(wqk_t[p], wqk_d[p, :, :])
            for dc in range(NDC):
                nc.sync.dma_start(wv_sb[dc], wv_d[dc, :, :])
            nc.sync.dma_start(
                bv_b, bass.AP(tensor=bv_d, offset=0, ap=[[0, 128], [1, D]])
            )
            nc.sync.dma_start(
                bo_b, bass.AP(tensor=bo_d, offset=0, ap=[[0, 128], [1, D]])
            )
            nc.sync.dma_start(ident, ident_d[:, :])
            for dc in range(NDC):
                nc.sync.dma_start(wo_sb[dc], wo_d[dc, :, :])

            # persistent V tiles: [t 128, 12*65] (col h*65+64 is the ones col)
            VW = H * 65  # 780
            v_t = [vpool.tile([128, VW], BF16, name=f"V{t}") for t in range(NTC)]
            # persistent OT tiles per pair: [of 128, 1024 s]
            OT = [ot_pool.tile([128, S], BF16, name=f"OT{p}") for p in range(NPAIR)]
            # partial y (pairs 0..4 + bias), per s-chunk, bf16
            ypart = yp_pool.tile([128, NSC * D], BF16, name="ypart")

            warm = const.tile([128, 512], BF16, name="warm")
            nc.vector.memset(warm, 0.0)
            for i in range(5):
                wps = ps.tile([128, 512], F32, tag="pp", bufs=2, name=f"warm{i}")
                nc.tensor.matmul(
                    wps[:, 0:256], warm[:, 0:128], warm[:, 0:256],
                    start=True, stop=True,
                )

            q8 = {}
            k8 = {}
            pts = {}  # pts[p][h2][tcb] -> P^T tile

            # ---------- chunk emitters (each ~0.4-1.6us of PE work) ----------

            proj_pst = {}

            def proj_chunk(p, which, nh, part=None):
                """One 512-col half of a q/k projection + its evacuation.

                part=0: first 3 d-chunks; part=1: last 3 + evac; None: all.
                """
                w = wq_sb[p] if which == "q" else wk_sb[p]
                if part == 1:
                    pst = proj_pst.pop((p, which, nh))
                else:
                    pst = ps.tile(
                        [128, 512], F32, tag="pp", bufs=2, name=f"pj{p}{which}{nh}"
                    )
                rng = {None: range(NDC), 0: range(3), 1: range(3, NDC)}[part]
                for dc in rng:
                    nc.tensor.matmul(
                        pst,
                        w[:, dc * 128 : (dc + 1) * 128],
                        xT[dc][:, nh * 512 : (nh + 1) * 512],
                        start=(dc == 0),
                        stop=(dc == NDC - 1),
                    )
                if part == 0:
                    proj_pst[(p, which, nh)] = pst
                    return
                sl = slice(nh * 512, (nh + 1) * 512)
                with tc.high_priority(offset=200):
                    if which == "q":
                        hi = q8[p][:, 0:S]
                        lo = q8[p][:, S : 2 * S]
                        nc.vector.tensor_scalar_add(
                            hi[:, sl], pst, bq_sb[:, p : p + 1]
                        )
                        nc.vector.scalar_tensor_tensor(
                            lo[:, sl], pst, bq_sb[:, p : p + 1], hi[:, sl],
                            Alu.add, Alu.subtract,
                        )
                    else:
                        nc.vector.tensor_copy(k8[p][:, sl], pst)

            def score_chunk(p, h2, tcb, split_exp=False):
                """One scores tile (2 fp8-DR matmuls) + exp -> P^T tile.

                split_exp: exp per s-half right behind its matmul — used for
                the first tiles so ACT starts before the q-nh1 evac lands.
                """
                q3 = q8[p].rearrange("p (two s) -> p two s", two=2)
                psl = slice(h2 * 64, (h2 + 1) * 64)
                st = ps.tile([128, S], F32, tag="sc", bufs=2, name=f"s{p}{h2}{tcb}")
                lhsT = (
                    k8[p][psl, tcb * 128 : (tcb + 1) * 128]
                    .unsqueeze(1)
                    .broadcast_to((64, 2, 128))
                )
                pt = pt_pool.tile([128, S], BF16, tag="pt", name=f"pt{p}{h2}{tcb}")
                for sh in range(2):
                    ssl = slice(sh * 512, (sh + 1) * 512)
                    nc.tensor.matmul(
                        st[:, ssl],
                        lhsT,
                        q3[psl, :, ssl],
                        start=True,
                        stop=True,
                        perf_mode=DR,
                    )
                    if split_exp:
                        nc.scalar.activation(pt[:, ssl], st[:, ssl], Exp, scale=0.125)
                if not split_exp:
                    nc.scalar.activation(pt, st, Exp, scale=0.125)
                pts[p][h2][tcb] = pt

            def v_chunk(tcb, half):
                """v-proj for one (t-chunk, col-half); swapped operands."""
                c0, c1 = ((0, 512), (512, 768))[half]
                n = c1 - c0
                pv = ps.tile([128, 512], F32, tag="pp", bufs=2, name=f"vv{tcb}{half}")
                for dc in range(NDC):
                    nc.tensor.matmul(
                        pv[:, 0:n],
                        xT[dc][:, tcb * 128 : (tcb + 1) * 128],
                        wv_sb[dc][:, c0:c1],
                        start=(dc == 0),
                        stop=(dc == NDC - 1),
                    )
                nh = n // 64
                h0 = c0 // 64
                dst = v_t[tcb][:, h0 * 65 : h0 * 65 + nh * 65]
                dst3 = dst.rearrange("p (h e) -> p h e", e=65)[:, :, 0:64]
                src3 = pv[:, 0:n].rearrange("p (h e) -> p h e", e=64)
                bias3 = bv_b[:, c0:c1].rearrange("p (h e) -> p h e", e=64)
                nc.vector.tensor_tensor(dst3, src3, bias3, Alu.add)
                if half == 1:
                    ones3 = v_t[tcb].rearrange("p (h e) -> p h e", e=65)[:, :, 64:65]
                    nc.vector.memset(ones3, 1.0)

            onrm_t = {}

            av_ov = {}

            def av_chunk(p, sc, heads=(0, 1), transpose="dma", quarter=None):
                """O for one (pair, s-chunk): AV matmuls, normalize, transpose.

                transpose: "dma" (SP hwdge queue), "pe" (tail: PE + Pool copy),
                or None (first-head half of a split pair).
                """
                if (p, sc) in onrm_t:
                    onrm = onrm_t.pop((p, sc))
                else:
                    onrm = on_pool.tile(
                        [128, 128], BF16, tag="on", name=f"on{p}{sc}"
                    )
                if quarter is not None:
                    heads = (quarter // 2,)
                    tcbs = range(4) if quarter % 2 == 0 else range(4, NTC)
                else:
                    tcbs = range(NTC)
                for h2 in heads:
                    h = 2 * p + h2
                    if quarter is not None and quarter % 2 == 1:
                        ov = av_ov.pop((p, sc, h2))
                    else:
                        ov = ps.tile(
                            [128, 65], F32, tag="ov", bufs=2, name=f"ov{p}{sc}{h2}"
                        )
                    for tcb in tcbs:
                        nc.tensor.matmul(
                            ov,
                            pts[p][h2][tcb][:, sc * 128 : (sc + 1) * 128],
                            v_t[tcb][:, h * 65 : h * 65 + 65],
                            start=(tcb == 0),
                            stop=(tcb == NTC - 1),
                        )
                    if quarter is not None and quarter % 2 == 0:
                        av_ov[(p, sc, h2)] = ov
                        onrm_t[(p, sc)] = onrm
                        return
                    rsb = ov_sb_pool.tile(
                        [128, 1], F32, tag="ovsb", name=f"r{p}{sc}{h2}"
                    )
                    import contextlib as _cl
                    hp = tc.high_priority(offset=250) if p == 5 else _cl.nullcontext()
                    with hp:
                        nc.vector.reciprocal(out=rsb, in_=ov[:, 64:65])
                        nc.vector.tensor_scalar_mul(
                            onrm[:, h2 * 64 : (h2 + 1) * 64], ov[:, 0:64], rsb
                        )
                if quarter is not None and quarter < 3:
                    onrm_t[(p, sc)] = onrm
                    return
                if transpose is None:
                    onrm_t[(p, sc)] = onrm
                elif transpose == "dma":
                    nc.sync.dma_start_transpose(
                        OT[p][:, sc * 128 : (sc + 1) * 128], onrm
                    )
                else:
                    tp = ps.tile([128, 128], BF16, tag="ov", bufs=2, name=f"tp{sc}")
                    nc.tensor.transpose(tp, onrm, ident)
                    with tc.high_priority(offset=250):
                        nc.vector.tensor_copy(
                            OT[p][:, sc * 128 : (sc + 1) * 128], tp
                        )

            def ypart_chunk(sc, half=None):
                """Out-proj partial: pairs 0..4 + bias -> bf16 ypart."""
                cols = ((0, 512), (512, 768))
                if half is not None:
                    cols = (cols[half],)
                for c0, c1 in cols:
                    n = c1 - c0
                    yp = ps.tile([128, 512], F32, tag="pp", bufs=2, name=f"yp{sc}")
                    for p in range(NPAIR - 1):
                        nc.tensor.matmul(
                            yp[:, 0:n],
                            OT[p][:, sc * 128 : (sc + 1) * 128],
                            wo_sb[p][:, c0:c1],
                            start=(p == 0),
                            stop=(p == NPAIR - 2),
                        )
                    nc.vector.tensor_add(
                        ypart[:, sc * D + c0 : sc * D + c1], yp[:, 0:n],
                        bo_b[:, c0:c1],
                    )

            def yfinal_chunk(sc):
                """Fold pair 5 + the bf16 partial (identity matmul), DMA out
                straight from PSUM."""
                yf = ps.tile([128, D], F32, tag="sc", bufs=2, name=f"yf{sc}")
                lhsT = OT[NPAIR - 1][:, sc * 128 : (sc + 1) * 128]
                for c0, c1 in ((0, 512), (512, 768)):
                    nc.tensor.matmul(
                        yf[:, c0:c1], lhsT, wo_sb[NPAIR - 1][:, c0:c1],
                        start=True, stop=False,
                    )
                    nc.tensor.matmul(
                        yf[:, c0:c1], ident,
                        ypart[:, sc * D + c0 : sc * D + c1],
                        start=False, stop=True,
                    )
                ysb = y_pool.tile([128, D], F32, tag="y", name=f"y{sc}")
                if sc % 2 == 0:
                    with tc.high_priority(offset=250):
                        nc.vector.tensor_copy(ysb, yf)
                else:
                    nc.scalar.copy(ysb, yf)
                eng = (nc.sync, nc.scalar)[sc % 2]
                eng.dma_start(y_d[sc * 128 : (sc + 1) * 128, :], ysb)

            # ---------- interleaved schedule ----------
            # Filler work (est PE ns, thunk) emitted between score chunks so
            # the in-order PE stream never blocks on the scores/exp ping-pong.

            def alloc_pair(p):
                q8[p] = qk8.tile([128, 2 * S], FP8, tag="q8", name=f"q8_{p}")
                k8[p] = qk8.tile([128, S], FP8, tag="k8", name=f"k8_{p}")
                pts[p] = [[None] * NTC for _ in range(2)]

            def proj_fillers(p):
                def mk(which, nh, part):
                    return lambda: proj_chunk(p, which, nh, part)

                out = []
                for which, nh in (("q", 0), ("k", 0), ("q", 1), ("k", 1)):
                    out.append((680, mk(which, nh, 0)))
                    out.append((680, mk(which, nh, 1)))
                return out

            def av_fillers(p):
                return [
                    (480, (lambda sc_: lambda: av_chunk(p, sc_))(sc))
                    for sc in range(NSC)
                ]

            def av_fillers_q(p):
                out = []
                for sc in range(NSC):
                    for q in range(4):
                        out.append(
                            (150,
                             (lambda sc_, q_: lambda: av_chunk(p, sc_, quarter=q_))(sc, q))
                        )
                return out

            v_fillers = [
                (1350 if half == 0 else 700,
                 (lambda t_, h_: lambda: v_chunk(t_, h_))(tcb, half))
                for tcb in range(NTC)
                for half in range(2)
            ]

            def av5_h0_fillers():
                out = []
                for sc in range(NSC):
                    for q in (0, 1):
                        out.append(
                            (130,
                             (lambda sc_, q_: lambda: av_chunk(
                                 5, sc_, quarter=q_, transpose=None))(sc, q))
                        )
                return out

            ypart_fillers = [
                (1100 if h == 0 else 580,
                 (lambda sc_, h_: lambda: ypart_chunk(sc_, h_))(sc, h))
                for sc in range(NSC)
                for h in range(2)
            ]

            fillers_by_pair = {
                0: proj_fillers(1),
                1: v_fillers[:8] + proj_fillers(2),
                2: v_fillers[8:] + proj_fillers(3) + av_fillers(0),
                3: proj_fillers(4) + av_fillers(1),
                4: proj_fillers(5) + av_fillers(2) + av_fillers_q(3),
                5: av_fillers_q(4) + av5_h0_fillers() + ypart_fillers,
            }

            alloc_pair(0)
            proj_chunk(0, "q", 0)
            proj_chunk(0, "k", 0)
            # first two score tiles, s-half 0 only (q-nh1 not needed yet):
            # emitted as split matmuls so ACT starts ~2us earlier.
            intro_st = {}
            q3_0 = q8[0].rearrange("p (two s) -> p two s", two=2)
            for tcb in range(2):
                st = ps.tile([128, S], F32, tag="sc", bufs=2, name=f"i{tcb}")
                pt = pt_pool.tile([128, S], BF16, tag="pt", name=f"ipt{tcb}")
                lhsT = (
                    k8[0][0:64, tcb * 128 : (tcb + 1) * 128]
                    .unsqueeze(1)
                    .broadcast_to((64, 2, 128))
                )
                nc.tensor.matmul(
                    st[:, 0:512], lhsT, q3_0[0:64, :, 0:512],
                    start=True, stop=True, perf_mode=DR,
                )
                nc.scalar.activation(pt[:, 0:512], st[:, 0:512], Exp, scale=0.125)
                intro_st[tcb] = (st, pt, lhsT)
            proj_chunk(0, "q", 1)
            for tcb in range(2):
                st, pt, lhsT = intro_st[tcb]
                nc.tensor.matmul(
                    st[:, 512:S], lhsT, q3_0[0:64, :, 512:S],
                    start=True, stop=True, perf_mode=DR,
                )
                nc.scalar.activation(pt[:, 512:S], st[:, 512:S], Exp, scale=0.125)
                pts[0][0][tcb] = pt
            fillers_by_pair[0] = [(680, lambda: proj_chunk(0, "k", 1))] + \
                fillers_by_pair[0]

            FILL_NS = 580  # target filler PE-ns per score chunk
            for p in range(NPAIR):
                if p + 1 < NPAIR:
                    alloc_pair(p + 1)
                fill = list(fillers_by_pair[p])
                fi = 0
                for h2 in range(2):
                    for tcb in range(NTC):
                        if p == 0 and h2 == 0 and tcb < 2:
                            continue  # emitted in the intro above
                        score_chunk(p, h2, tcb)
                        budget = FILL_NS
                        while fi < len(fill) and budget > 0:
                            est, thunk = fill[fi]
                            thunk()
                            budget -= est
                            fi += 1
                while fi < len(fill):
                    fill[fi][1]()
                    fi += 1
                if p >= 2:
                    del pts[p - 2]

            # tail: pair-5 head-11 AV per s-chunk (head 10 was a filler),
            # PE-transpose + Pool copy (no DMA queue latency), then the final
            # out-proj chunk for that s-chunk.
            for sc in range(NSC):
                av_chunk(5, sc, heads=(1,), transpose="pe")
                yfinal_chunk(sc)

    nc.compile()
    return nc


def _prep_inputs(x, Wq, bq, Wk, bk, Wv, bv, Wo, bo):
    """Host-side layout transforms + bf16 casts."""
    x = np.asarray(x)
    xT = np.ascontiguousarray(x.transpose(0, 2, 1)).astype(_BF16)

    def pack_w(W):
        # W [H, D, DH] -> [NPAIR, 128(d_sub), D] where col dc*128+m holds
        # W[2p + m//64, dc*128+d_sub, m%64]
        Wp = np.empty((NPAIR, 128, D), np.float32)
        W = np.asarray(W, np.float32)
        for p in range(NPAIR):
            blk = np.concatenate([W[2 * p], W[2 * p + 1]], axis=1)  # [D, 128]
            Wp[p] = blk.reshape(NDC, 128, 128).transpose(1, 0, 2).reshape(128, D)
        return Wp

    wqk = np.concatenate([pack_w(Wq), pack_w(Wk)], axis=2).astype(_BF16)

    bq_sb = np.empty((128, NPAIR), np.float32)
    bq = np.asarray(bq, np.float32)
    for p in range(NPAIR):
        bq_sb[:, p] = np.concatenate([bq[2 * p], bq[2 * p + 1]])

    # Wv natural [D, H*DH] chunked over d
    Wv = np.asarray(Wv, np.float32)
    wv = Wv.transpose(1, 0, 2).reshape(D, D).reshape(NDC, 128, D).astype(_BF16)
    Wo = np.asarray(Wo, np.float32)
    wo = Wo.reshape(NDC, 128, D).astype(_BF16)

    bv_h = np.asarray(bv, np.float32).reshape(1, D)
    bo_h = np.asarray(bo, np.float32).reshape(1, D)

    shared = {
        "wqk": wqk,
        "wv": wv,
        "wo": wo,
        "bq": bq_sb,
        "bv": bv_h,
        "bo": bo_h,
        "ident": np.eye(128, dtype=np.float32).astype(_BF16),
    }
    return xT, shared


def kernel(x, Wq, bq, Wk, bk, Wv, bv, Wo, bo):
    from concourse.bass_utils import run_bass_kernel_spmd

    if "nc" not in _cache:
        _cache["nc"] = _build_program()
    nc = _cache["nc"]

    xT, shared = _prep_inputs(x, Wq, bq, Wk, bk, Wv, bv, Wo, bo)
    in_maps = [dict(shared, xT=np.ascontiguousarray(xT[b])) for b in range(B)]
    res = run_bass_kernel_spmd(nc, in_maps, core_ids=list(range(B)))
    y = np.stack([res.results[b]["y"] for b in range(B)], axis=0)
    return y.astype(np.float32)


# revision 5
# speedup vs baseline: 1.0306x; 1.0085x over previous
"""Multi-head attention (B=8, S=1024, D=768, H=12, DH=64) on 8 TRN2 NeuronCores.

Strategy: pure data parallelism over batch — core b computes batch element b
end-to-end (no collectives). Optimized v2:

  - q/k projections (bf16, PE): qT/kT [128(2 heads' e), 1024] per pair.
    q evacuated straight to fp8 hi/lo pair (DVE), k straight to fp8 (Pool).
    bk dropped entirely (softmax-invariant).
  - v projection operand-swapped (bf16, PE): lhsT = xT chunks (stationary),
    rhs = Wv natural (moving) -> V [t, e] natural tiles directly, with a
    ones-column per head appended for free denominators.
  - scores: fp8e4m3 DoubleRow matmuls (0.5 cyc/row): lhsT = k8 broadcast
    (two identical k-tiles, stride 0), rhs = (q_hi, q_lo) -> computes
    k.T @ (q_hi + q_lo): q at ~bf16 precision, k plain fp8. rel err ~5e-3.
  - exp on ACT -> P^T bf16 tiles (the only ACT work, ~100us = the wall).
  - AV operand-swapped: P^T chunks stationary, [V | 1] moving: cost counts
    only moving columns -> 2x cheaper than the m=65 orientation, and the
    denominator lands as a per-partition column -> normalize is a cheap
    per-partition tensor_scalar divide (Pool), no DRAM broadcast roundtrip.
  - O^T via DMA transpose (SBUF->SBUF, no PE/PSUM cost).
  - out-proj split: pairs 0-4 accumulated into a bf16 partial during the
    last pair's exp window; pair 5 folded in per s-chunk at the tail.
  - emission is software-pipelined at ~0.5us granularity: after each score
    tile (2 matmuls + exp) the emitter appends independent PE work (next
    pair's projections, v-proj, AV of pair p-2, out-proj partials) so the
    in-order PE queue never blocks on the scores/exp PSUM ping-pong.

All matmul inputs bf16/fp8 (fp32 PSUM accumulation); output f32.
"""

import sys

sys.path.insert(0, "/opt/trn_rl_repo")

import numpy as np
import ml_dtypes

B, S, D = 8, 1024, 768
H = 12
DH = 64
NPAIR = 6  # head pairs
NDC = 6  # 128-wide chunks of D
NTC = 8  # 128-wide chunks of S (key/t side)
NSC = 8  # 128-wide chunks of S (query/s side)

_BF16 = ml_dtypes.bfloat16
_FP8 = ml_dtypes.float8_e4m3fn

_cache = {}


def _build_program():
    import concourse.bass as bass
    import concourse.bacc as bacc
    import concourse.tile as tile
    from concourse import mybir

    F32 = mybir.dt.float32
    BF16 = mybir.dt.bfloat16
    FP8 = mybir.dt.float8e4
    Exp = mybir.ActivationFunctionType.Exp
    Alu = mybir.AluOpType
    DR = mybir.MatmulPerfMode.DoubleRow

    nc = bacc.Bacc("TRN2", target_bir_lowering=False, debug=False)

    # ---- DRAM I/O (per core) ----
    xT_d = nc.dram_tensor("xT", [D, S], BF16, kind="ExternalInput")
    wqk_d = nc.dram_tensor("wqk8", [NPAIR, 128, 2 * D], FP8, kind="ExternalInput")
    x8_d = nc.dram_tensor("x8", [D, 2 * S], FP8, kind="ExternalInput")
    wv_d = nc.dram_tensor("wv", [NDC, 128, D], BF16, kind="ExternalInput")
    wo_d = nc.dram_tensor("wo", [NDC, 128, D], BF16, kind="ExternalInput")
    bq_d = nc.dram_tensor("bq", [128, NPAIR], F32, kind="ExternalInput")
    bv_d = nc.dram_tensor("bv", [1, D], F32, kind="ExternalInput")
    bo_d = nc.dram_tensor("bo", [1, D], F32, kind="ExternalInput")
    ident_d = nc.dram_tensor("ident", [128, 128], BF16, kind="ExternalInput")
    y_d = nc.dram_tensor("y", [S, D], F32, kind="ExternalOutput")

    with tile.TileContext(nc) as tc:
        import contextlib

        ctx = contextlib.ExitStack()
        with ctx:
            const = ctx.enter_context(tc.tile_pool(name="const", bufs=1))
            wpool = ctx.enter_context(tc.tile_pool(name="wpool", bufs=1))
            qk8 = ctx.enter_context(tc.tile_pool(name="qk8", bufs=3))
            vpool = ctx.enter_context(tc.tile_pool(name="vp", bufs=1))
            pt_pool = ctx.enter_context(tc.tile_pool(name="pt", bufs=42))
            on_pool = ctx.enter_context(tc.tile_pool(name="on", bufs=12))
            ov_sb_pool = ctx.enter_context(tc.tile_pool(name="ovsb", bufs=4))
            ot_pool = ctx.enter_context(tc.tile_pool(name="ot", bufs=1))
            yp_pool = ctx.enter_context(tc.tile_pool(name="ypart", bufs=1))
            y_pool = ctx.enter_context(tc.tile_pool(name="ysb", bufs=4))
            ps = ctx.enter_context(tc.tile_pool(name="ps", bufs=1, space="PSUM"))

            # ---- input DMAs; critical path first (pair-0 qk weights + x) ----
            xt_all = wpool.tile([128, NDC * S], BF16, name="xt_all")
            xT = [xt_all[:, dc * S : (dc + 1) * S] for dc in range(NDC)]
            xt_src = xT_d.rearrange("(dc p) s -> p dc s", p=128)

            wqk_t = {}
            for p in range(NPAIR):
                wqk_t[p] = wpool.tile([128, 2 * D], FP8, name=f"wqk{p}")
            wq_sb = {p: wqk_t[p][:, 0:D] for p in range(NPAIR)}
            wk_sb = {p: wqk_t[p][:, D : 2 * D] for p in range(NPAIR)}
            x8_all = wpool.tile([128, NDC * 2 * S], FP8, name="x8_all")
            # per d-chunk: [128, 2, 1024] = (hi cols 0:1024, lo cols 1024:2048)
            x8 = [
                x8_all[:, dc * 2 * S : (dc + 1) * 2 * S].rearrange(
                    "p (two s) -> p two s", two=2
                )
                for dc in range(NDC)
            ]
            x8_src = x8_d.rearrange("(dc p) s -> p dc s", p=128)

            # two hwdge queues (SP + ACT): critical-path tensors first
            bq_sb = const.tile([128, NPAIR], F32)
            wv_all = wpool.tile([128, NDC * D], BF16, name="wv_all")
            wv_sb = [wv_all[:, dc * D : (dc + 1) * D] for dc in range(NDC)]
            bv_b = const.tile([128, D], F32)
            bo_b = const.tile([128, D], F32)
            ident = const.tile([128, 128], BF16)
            wo_all = wpool.tile([128, NDC * D], BF16, name="wo_all")
            wo_sb = [wo_all[:, dc * D : (dc + 1) * D] for dc in range(NDC)]

            nc.sync.dma_start(wqk_t[0][:, 0:D], wqk_d[0, :, 0:D])
            nc.scalar.dma_start(bq_sb, bq_d[:, :])
            nc.scalar.dma_start(wqk_t[0][:, D : 2 * D], wqk_d[0, :, D : 2 * D])
            x8_dst = x8_all.rearrange("p (dc two s) -> p dc two s", dc=NDC, two=2)
            for dc in range(NDC):
                eng = (nc.sync, nc.scalar)[dc % 2]
                eng.dma_start(x8_dst[:, dc, :, :],
                              x8_src[:, dc, :].rearrange("p (two s) -> p two s", two=2))
            for dc in range(NDC):
                eng = (nc.sync, nc.scalar)[dc % 2]
                eng.dma_start(xT[dc], xt_src[:, dc, :])
            for p in range(1, NPAIR):
                nc.sync.dma_start(wqk_t[p], wqk_d[p, :, :])
            for dc in range(NDC):
                nc.sync.dma_start(wv_sb[dc], wv_d[dc, :, :])
            nc.sync.dma_start(
                bv_b, bass.AP(tensor=bv_d, offset=0, ap=[[0, 128], [1, D]])
            )
            nc.sync.dma_start(
                bo_b, bass.AP(tensor=bo_d, offset=0, ap=[[0, 128], [1, D]])
            )
            nc.sync.dma_start(ident, ident_d[:, :])
            for dc in range(NDC):
                nc.sync.dma_start(wo_sb[dc], wo_d[dc, :, :])

            # persistent V tiles: [t 128, 12*65] (col h*65+64 is the ones col)
            VW = H * 65  # 780
            v_t = [vpool.tile([128, VW], BF16, name=f"V{t}") for t in range(NTC)]
            # persistent OT tiles per pair: [of 128, 1024 s]
            OT = [ot_pool.tile([128, S], BF16, name=f"OT{p}") for p in range(NPAIR)]
            # partial y (pairs 0..4 + bias), per s-chunk, bf16
            ypart = yp_pool.tile([128, NSC * D], BF16, name="ypart")

            warm = const.tile([128, 512], BF16, name="warm")
            nc.vector.memset(warm, 0.0)
            for i in range(5):
                wps = ps.tile([128, 512], F32, tag="pp", bufs=2, name=f"warm{i}")
                nc.tensor.matmul(
                    wps[:, 0:256], warm[:, 0:128], warm[:, 0:256],
                    start=True, stop=True,
                )

            q8 = {}
            k8 = {}
            pts = {}  # pts[p][h2][tcb] -> P^T tile

            # ---------- chunk emitters (each ~0.4-1.6us of PE work) ----------

            proj_pst = {}

            def proj_chunk(p, which, nh, part=None):
                """One 512-col half of a q/k projection + its evacuation.

                part=0: first 3 d-chunks; part=1: last 3 + evac; None: all.
                """
                w = wq_sb[p] if which == "q" else wk_sb[p]
                if part == 1:
                    pst = proj_pst.pop((p, which, nh))
                else:
                    pst = ps.tile(
                        [128, 512], F32, tag="pp", bufs=2, name=f"pj{p}{which}{nh}"
                    )
                rng = {None: range(NDC), 0: range(3), 1: range(3, NDC)}[part]
                for dc in rng:
                    nc.tensor.matmul(
                        pst,
                        w[:, dc * 128 : (dc + 1) * 128]
                        .unsqueeze(1)
                        .broadcast_to((128, 2, 128)),
                        x8[dc][:, :, nh * 512 : (nh + 1) * 512],
                        start=(dc == 0),
                        stop=(dc == NDC - 1),
                        perf_mode=DR,
                    )
                if part == 0:
                    proj_pst[(p, which, nh)] = pst
                    return
                sl = slice(nh * 512, (nh + 1) * 512)
                with tc.high_priority(offset=200):
                    if which == "q":
                        hi = q8[p][:, 0:S]
                        lo = q8[p][:, S : 2 * S]
                        nc.vector.tensor_scalar_add(
                            hi[:, sl], pst, bq_sb[:, p : p + 1]
                        )
                        nc.vector.scalar_tensor_tensor(
                            lo[:, sl], pst, bq_sb[:, p : p + 1], hi[:, sl],
                            Alu.add, Alu.subtract,
                        )
                    else:
                        nc.vector.tensor_copy(k8[p][:, sl], pst)

            def score_chunk(p, h2, tcb, split_exp=False):
                """One scores tile (2 fp8-DR matmuls) + exp -> P^T tile.

                split_exp: exp per s-half right behind its matmul — used for
                the first tiles so ACT starts before the q-nh1 evac lands.
                """
                q3 = q8[p].rearrange("p (two s) -> p two s", two=2)
                psl = slice(h2 * 64, (h2 + 1) * 64)
                st = ps.tile([128, S], F32, tag="sc", bufs=2, name=f"s{p}{h2}{tcb}")
                lhsT = (
                    k8[p][psl, tcb * 128 : (tcb + 1) * 128]
                    .unsqueeze(1)
                    .broadcast_to((64, 2, 128))
                )
                pt = pt_pool.tile([128, S], BF16, tag="pt", name=f"pt{p}{h2}{tcb}")
                for sh in range(2):
                    ssl = slice(sh * 512, (sh + 1) * 512)
                    nc.tensor.matmul(
                        st[:, ssl],
                        lhsT,
                        q3[psl, :, ssl],
                        start=True,
                        stop=True,
                        perf_mode=DR,
                    )
                    if split_exp:
                        nc.scalar.activation(pt[:, ssl], st[:, ssl], Exp, scale=0.125)
                if not split_exp:
                    nc.scalar.activation(pt, st, Exp, scale=0.125)
                pts[p][h2][tcb] = pt

            def v_chunk(tcb, half):
                """v-proj for one (t-chunk, col-half); swapped operands."""
                c0, c1 = ((0, 512), (512, 768))[half]
                n = c1 - c0
                pv = ps.tile([128, 512], F32, tag="pp", bufs=2, name=f"vv{tcb}{half}")
                for dc in range(NDC):
                    nc.tensor.matmul(
                        pv[:, 0:n],
                        xT[dc][:, tcb * 128 : (tcb + 1) * 128],
                        wv_sb[dc][:, c0:c1],
                        start=(dc == 0),
                        stop=(dc == NDC - 1),
                    )
                nh = n // 64
                h0 = c0 // 64
                dst = v_t[tcb][:, h0 * 65 : h0 * 65 + nh * 65]
                dst3 = dst.rearrange("p (h e) -> p h e", e=65)[:, :, 0:64]
                src3 = pv[:, 0:n].rearrange("p (h e) -> p h e", e=64)
                bias3 = bv_b[:, c0:c1].rearrange("p (h e) -> p h e", e=64)
                nc.vector.tensor_tensor(dst3, src3, bias3, Alu.add)
                if half == 1:
                    ones3 = v_t[tcb].rearrange("p (h e) -> p h e", e=65)[:, :, 64:65]
                    nc.vector.memset(ones3, 1.0)

            onrm_t = {}

            av_ov = {}

            def av_chunk(p, sc, heads=(0, 1), transpose="dma", quarter=None):
                """O for one (pair, s-chunk): AV matmuls, normalize, transpose.

                transpose: "dma" (SP hwdge queue), "pe" (tail: PE + Pool copy),
                or None (first-head half of a split pair).
                """
                if (p, sc) in onrm_t:
                    onrm = onrm_t.pop((p, sc))
                else:
                    onrm = on_pool.tile(
                        [128, 128], BF16, tag="on", name=f"on{p}{sc}"
                    )
                if quarter is not None:
                    heads = (quarter // 2,)
                    tcbs = range(4) if quarter % 2 == 0 else range(4, NTC)
                else:
                    tcbs = range(NTC)
                for h2 in heads:
                    h = 2 * p + h2
                    if quarter is not None and quarter % 2 == 1:
                        ov = av_ov.pop((p, sc, h2))
                    else:
                        ov = ps.tile(
                            [128, 65], F32, tag="ov", bufs=2, name=f"ov{p}{sc}{h2}"
                        )
                    for tcb in tcbs:
                        nc.tensor.matmul(
                            ov,
                            pts[p][h2][tcb][:, sc * 128 : (sc + 1) * 128],
                            v_t[tcb][:, h * 65 : h * 65 + 65],
                            start=(tcb == 0),
                            stop=(tcb == NTC - 1),
                        )
                    if quarter is not None and quarter % 2 == 0:
                        av_ov[(p, sc, h2)] = ov
                        onrm_t[(p, sc)] = onrm
                        return
                    rsb = ov_sb_pool.tile(
                        [128, 1], F32, tag="ovsb", name=f"r{p}{sc}{h2}"
                    )
                    import contextlib as _cl
                    hp = tc.high_priority(offset=250) if p == 5 else _cl.nullcontext()
                    with hp:
                        nc.vector.reciprocal(out=rsb, in_=ov[:, 64:65])
                        nc.vector.tensor_scalar_mul(
                            onrm[:, h2 * 64 : (h2 + 1) * 64], ov[:, 0:64], rsb
                        )
                if quarter is not None and quarter < 3:
                    onrm_t[(p, sc)] = onrm
                    return
                if transpose is None:
                    onrm_t[(p, sc)] = onrm
                elif transpose == "dma":
                    nc.sync.dma_start_transpose(
                        OT[p][:, sc * 128 : (sc + 1) * 128], onrm
                    )
                else:
                    tp = ps.tile([128, 128], BF16, tag="ov", bufs=2, name=f"tp{sc}")
                    nc.tensor.transpose(tp, onrm, ident)
                    with tc.high_priority(offset=250):
                        nc.vector.tensor_copy(
                            OT[p][:, sc * 128 : (sc + 1) * 128], tp
                        )

            def ypart_chunk(sc, half=None):
                """Out-proj partial: pairs 0..4 + bias -> bf16 ypart."""
                cols = ((0, 512), (512, 768))
                if half is not None:
                    cols = (cols[half],)
                for c0, c1 in cols:
                    n = c1 - c0
                    yp = ps.tile([128, 512], F32, tag="pp", bufs=2, name=f"yp{sc}")
                    for p in range(NPAIR - 1):
                        nc.tensor.matmul(
                            yp[:, 0:n],
                            OT[p][:, sc * 128 : (sc + 1) * 128],
                            wo_sb[p][:, c0:c1],
                            start=(p == 0),
                            stop=(p == NPAIR - 2),
                        )
                    nc.vector.tensor_add(
                        ypart[:, sc * D + c0 : sc * D + c1], yp[:, 0:n],
                        bo_b[:, c0:c1],
                    )

            def yfinal_chunk(sc):
                """Fold pair 5 + the bf16 partial (identity matmul), DMA out
                straight from PSUM."""
                yf = ps.tile([128, D], F32, tag="sc", bufs=2, name=f"yf{sc}")
                lhsT = OT[NPAIR - 1][:, sc * 128 : (sc + 1) * 128]
                for c0, c1 in ((0, 512), (512, 768)):
                    nc.tensor.matmul(
                        yf[:, c0:c1], lhsT, wo_sb[NPAIR - 1][:, c0:c1],
                        start=True, stop=False,
                    )
                    nc.tensor.matmul(
                        yf[:, c0:c1], ident,
                        ypart[:, sc * D + c0 : sc * D + c1],
                        start=False, stop=True,
                    )
                ysb = y_pool.tile([128, D], F32, tag="y", name=f"y{sc}")
                if sc % 2 == 0:
                    with tc.high_priority(offset=250):
                        nc.vector.tensor_copy(ysb, yf)
                else:
                    nc.scalar.copy(ysb, yf)
                eng = (nc.sync, nc.scalar)[sc % 2]
                eng.dma_start(y_d[sc * 128 : (sc + 1) * 128, :], ysb)

            # ---------- interleaved schedule ----------
            # Filler work (est PE ns, thunk) emitted between score chunks so
            # the in-order PE stream never blocks on the scores/exp ping-pong.

            def alloc_pair(p):
                q8[p] = qk8.tile([128, 2 * S], FP8, tag="q8", name=f"q8_{p}")
                k8[p] = qk8.tile([128, S], FP8, tag="k8", name=f"k8_{p}")
                pts[p] = [[None] * NTC for _ in range(2)]

            def proj_fillers(p):
                def mk(which, nh, part):
                    return lambda: proj_chunk(p, which, nh, part)

                out = []
                for which, nh in (("q", 0), ("k", 0), ("q", 1), ("k", 1)):
                    out.append((340, mk(which, nh, 0)))
                    out.append((340, mk(which, nh, 1)))
                return out

            def av_fillers(p):
                return [
                    (480, (lambda sc_: lambda: av_chunk(p, sc_))(sc))
                    for sc in range(NSC)
                ]

            def av_fillers_q(p, interleave=None):
                out = []
                for sc in range(NSC):
                    for q in range(4):
                        out.append(
                            (150,
                             (lambda sc_, q_: lambda: av_chunk(
                                 p, sc_, quarter=q_))(sc, q))
                        )
                    if interleave is not None and sc >= 3:
                        out.extend(interleave(sc - 3))
                if interleave is not None:
                    for sc in (NSC - 3, NSC - 2, NSC - 1):
                        out.extend(interleave(sc))
                return out

            v_fillers = [
                (1350 if half == 0 else 700,
                 (lambda t_, h_: lambda: v_chunk(t_, h_))(tcb, half))
                for tcb in range(NTC)
                for half in range(2)
            ]

            def av5_h0_fillers():
                # head-10 AV split into half-chunks (4 mms each) so blocked
                # instrs fit the 4-deep PE wait queue.
                out = []
                for sc in range(NSC):
                    for q in (0, 1):
                        out.append(
                            (130,
                             (lambda sc_, q_: lambda: av_chunk(
                                 5, sc_, quarter=q_, transpose=None))(sc, q))
                        )
                return out

            def ypart_interleave(sc):
                return [
                    (1100 if h == 0 else 580,
                     (lambda sc_, h_: lambda: ypart_chunk(sc_, h_))(sc, h))
                    for h in range(2)
                ]

            fillers_by_pair = {
                0: proj_fillers(1),
                1: v_fillers[:8] + proj_fillers(2),
                2: v_fillers[8:] + proj_fillers(3) + av_fillers(0),
                3: proj_fillers(4) + av_fillers(1),
                4: proj_fillers(5) + av_fillers(2) + av_fillers_q(3),
                5: av_fillers_q(4) + av5_h0_fillers()
                   + [f for sc in range(NSC) for f in ypart_interleave(sc)],
            }

            alloc_pair(0)
            proj_chunk(0, "q", 0)
            proj_chunk(0, "k", 0)
            # first two score tiles, s-half 0 only (q-nh1 not needed yet):
            # emitted as split matmuls so ACT starts ~2us earlier.
            intro_st = {}
            q3_0 = q8[0].rearrange("p (two s) -> p two s", two=2)
            for tcb in range(2):
                st = ps.tile([128, S], F32, tag="sc", bufs=2, name=f"i{tcb}")
                pt = pt_pool.tile([128, S], BF16, tag="pt", name=f"ipt{tcb}")
                lhsT = (
                    k8[0][0:64, tcb * 128 : (tcb + 1) * 128]
                    .unsqueeze(1)
                    .broadcast_to((64, 2, 128))
                )
                nc.tensor.matmul(
                    st[:, 0:512], lhsT, q3_0[0:64, :, 0:512],
                    start=True, stop=True, perf_mode=DR,
                )
                nc.scalar.activation(pt[:, 0:512], st[:, 0:512], Exp, scale=0.125)
                intro_st[tcb] = (st, pt, lhsT)
            proj_chunk(0, "q", 1)
            for tcb in range(2):
                st, pt, lhsT = intro_st[tcb]
                nc.tensor.matmul(
                    st[:, 512:S], lhsT, q3_0[0:64, :, 512:S],
                    start=True, stop=True, perf_mode=DR,
                )
                nc.scalar.activation(pt[:, 512:S], st[:, 512:S], Exp, scale=0.125)
                pts[0][0][tcb] = pt
            fillers_by_pair[0] = [(680, lambda: proj_chunk(0, "k", 1))] + \
                fillers_by_pair[0]

            FILL_NS = 580  # target filler PE-ns per score chunk
            for p in range(NPAIR):
                if p + 1 < NPAIR:
                    alloc_pair(p + 1)
                fill = list(fillers_by_pair[p])
                fi = 0
                for h2 in range(2):
                    for tcb in range(NTC):
                        if p == 0 and h2 == 0 and tcb < 2:
                            continue  # emitted in the intro above
                        score_chunk(p, h2, tcb)
                        budget = FILL_NS
                        while fi < len(fill) and budget > 0:
                            est, thunk = fill[fi]
                            thunk()
                            budget -= est
                            fi += 1
                while fi < len(fill):
                    fill[fi][1]()
                    fi += 1
                if p >= 2:
                    del pts[p - 2]

            # tail: pair-5 head-11 AV per s-chunk (head 10 was a filler),
            # PE-transpose + Pool copy (no DMA queue latency), then the final
            # out-proj chunk for that s-chunk.
            for sc in range(NSC):
                av_chunk(5, sc, heads=(1,), transpose="pe")
                yfinal_chunk(sc)

    nc.compile()
    return nc


def _prep_inputs(x, Wq, bq, Wk, bk, Wv, bv, Wo, bo):
    """Host-side layout transforms + bf16 casts."""
    x = np.asarray(x)
    xT = np.ascontiguousarray(x.transpose(0, 2, 1)).astype(_BF16)
    xt_f32 = np.ascontiguousarray(x.transpose(0, 2, 1)).astype(np.float32)
    x8_hi = xt_f32.astype(_FP8)
    x8_lo = (xt_f32 - x8_hi.astype(np.float32)).astype(_FP8)
    # [B, D, 2, S] -> [B, D, 2*S]
    x8 = np.stack([x8_hi, x8_lo], axis=2).reshape(B, D, 2 * S)

    def pack_w(W):
        # W [H, D, DH] -> [NPAIR, 128(d_sub), D] where col dc*128+m holds
        # W[2p + m//64, dc*128+d_sub, m%64]
        Wp = np.empty((NPAIR, 128, D), np.float32)
        W = np.asarray(W, np.float32)
        for p in range(NPAIR):
            blk = np.concatenate([W[2 * p], W[2 * p + 1]], axis=1)  # [D, 128]
            Wp[p] = blk.reshape(NDC, 128, 128).transpose(1, 0, 2).reshape(128, D)
        return Wp

    wqk = np.concatenate([pack_w(Wq), pack_w(Wk)], axis=2).astype(_FP8)

    bq_sb = np.empty((128, NPAIR), np.float32)
    bq = np.asarray(bq, np.float32)
    for p in range(NPAIR):
        bq_sb[:, p] = np.concatenate([bq[2 * p], bq[2 * p + 1]])

    # Wv natural [D, H*DH] chunked over d
    Wv = np.asarray(Wv, np.float32)
    wv = Wv.transpose(1, 0, 2).reshape(D, D).reshape(NDC, 128, D).astype(_BF16)
    Wo = np.asarray(Wo, np.float32)
    wo = Wo.reshape(NDC, 128, D).astype(_BF16)

    bv_h = np.asarray(bv, np.float32).reshape(1, D)
    bo_h = np.asarray(bo, np.float32).reshape(1, D)

    shared = {
        "wqk8": wqk,
        "wv": wv,
        "wo": wo,
        "bq": bq_sb,
        "bv": bv_h,
        "bo": bo_h,
        "ident": np.eye(128, dtype=np.float32).astype(_BF16),
    }
    return xT, x8, shared


def kernel(x, Wq, bq, Wk, bk, Wv, bv, Wo, bo):
    from concourse.bass_utils import run_bass_kernel_spmd

    if "nc" not in _cache:
        _cache["nc"] = _build_program()
    nc = _cache["nc"]

    xT, x8, shared = _prep_inputs(x, Wq, bq, Wk, bk, Wv, bv, Wo, bo)
    in_maps = [
        dict(shared, xT=np.ascontiguousarray(xT[b]), x8=np.ascontiguousarray(x8[b]))
        for b in range(B)
    ]
    res = run_bass_kernel_spmd(nc, in_maps, core_ids=list(range(B)))
    y = np.stack([res.results[b]["y"] for b in range(B)], axis=0)
    return y.astype(np.float32)


# revision 6
# speedup vs baseline: 1.0343x; 1.0036x over previous
"""Multi-head attention (B=8, S=1024, D=768, H=12, DH=64) on 8 TRN2 NeuronCores.

Strategy: pure data parallelism over batch — core b computes batch element b
end-to-end (no collectives). Optimized v2:

  - q/k projections (bf16, PE): qT/kT [128(2 heads' e), 1024] per pair.
    q evacuated straight to fp8 hi/lo pair (DVE), k straight to fp8 (Pool).
    bk dropped entirely (softmax-invariant).
  - v projection operand-swapped (bf16, PE): lhsT = xT chunks (stationary),
    rhs = Wv natural (moving) -> V [t, e] natural tiles directly, with a
    ones-column per head appended for free denominators.
  - scores: fp8e4m3 DoubleRow matmuls (0.5 cyc/row): lhsT = k8 broadcast
    (two identical k-tiles, stride 0), rhs = (q_hi, q_lo) -> computes
    k.T @ (q_hi + q_lo): q at ~bf16 precision, k plain fp8. rel err ~5e-3.
  - exp on ACT -> P^T bf16 tiles (the only ACT work, ~100us = the wall).
  - AV operand-swapped: P^T chunks stationary, [V | 1] moving: cost counts
    only moving columns -> 2x cheaper than the m=65 orientation, and the
    denominator lands as a per-partition column -> normalize is a cheap
    per-partition tensor_scalar divide (Pool), no DRAM broadcast roundtrip.
  - O^T via DMA transpose (SBUF->SBUF, no PE/PSUM cost).
  - out-proj split: pairs 0-4 accumulated into a bf16 partial during the
    last pair's exp window; pair 5 folded in per s-chunk at the tail.
  - emission is software-pipelined at ~0.5us granularity: after each score
    tile (2 matmuls + exp) the emitter appends independent PE work (next
    pair's projections, v-proj, AV of pair p-2, out-proj partials) so the
    in-order PE queue never blocks on the scores/exp PSUM ping-pong.

All matmul inputs bf16/fp8 (fp32 PSUM accumulation); output f32.
"""

import sys

sys.path.insert(0, "/opt/trn_rl_repo")

import numpy as np
import ml_dtypes

B, S, D = 8, 1024, 768
H = 12
DH = 64
NPAIR = 6  # head pairs
NDC = 6  # 128-wide chunks of D
NTC = 8  # 128-wide chunks of S (key/t side)
NSC = 8  # 128-wide chunks of S (query/s side)

_BF16 = ml_dtypes.bfloat16
_FP8 = ml_dtypes.float8_e4m3fn

_cache = {}


def _build_program():
    import concourse.bass as bass
    import concourse.bacc as bacc
    import concourse.tile as tile
    from concourse import mybir

    F32 = mybir.dt.float32
    BF16 = mybir.dt.bfloat16
    FP8 = mybir.dt.float8e4
    Exp = mybir.ActivationFunctionType.Exp
    Alu = mybir.AluOpType
    DR = mybir.MatmulPerfMode.DoubleRow

    nc = bacc.Bacc("TRN2", target_bir_lowering=False, debug=False)

    # ---- DRAM I/O (per core) ----
    xT_d = nc.dram_tensor("xT", [D, S], BF16, kind="ExternalInput")
    wqk_d = nc.dram_tensor("wqk8", [NPAIR, 128, 2 * D], FP8, kind="ExternalInput")
    x8_d = nc.dram_tensor("x8", [D, 2 * S], FP8, kind="ExternalInput")
    wv_d = nc.dram_tensor("wv", [NDC, 128, D], BF16, kind="ExternalInput")
    wo_d = nc.dram_tensor("wo", [NDC, 128, D], BF16, kind="ExternalInput")
    bq_d = nc.dram_tensor("bq", [128, NPAIR], F32, kind="ExternalInput")
    bv_d = nc.dram_tensor("bv", [1, D], F32, kind="ExternalInput")
    bo_d = nc.dram_tensor("bo", [1, D], F32, kind="ExternalInput")
    ident_d = nc.dram_tensor("ident", [128, 128], BF16, kind="ExternalInput")
    y_d = nc.dram_tensor("y", [S, D], F32, kind="ExternalOutput")

    with tile.TileContext(nc) as tc:
        import contextlib

        ctx = contextlib.ExitStack()
        with ctx:
            const = ctx.enter_context(tc.tile_pool(name="const", bufs=1))
            wpool = ctx.enter_context(tc.tile_pool(name="wpool", bufs=1))
            qk8 = ctx.enter_context(tc.tile_pool(name="qk8", bufs=3))
            vpool = ctx.enter_context(tc.tile_pool(name="vp", bufs=1))
            pt_pool = ctx.enter_context(tc.tile_pool(name="pt", bufs=42))
            on_pool = ctx.enter_context(tc.tile_pool(name="on", bufs=12))
            ov_sb_pool = ctx.enter_context(tc.tile_pool(name="ovsb", bufs=4))
            ot_pool = ctx.enter_context(tc.tile_pool(name="ot", bufs=1))
            yp_pool = ctx.enter_context(tc.tile_pool(name="ypart", bufs=1))
            y_pool = ctx.enter_context(tc.tile_pool(name="ysb", bufs=4))
            ps = ctx.enter_context(tc.tile_pool(name="ps", bufs=1, space="PSUM"))

            # ---- input DMAs; critical path first (pair-0 qk weights + x) ----
            xt_all = wpool.tile([128, NDC * S], BF16, name="xt_all")
            xT = [xt_all[:, dc * S : (dc + 1) * S] for dc in range(NDC)]
            xt_src = xT_d.rearrange("(dc p) s -> p dc s", p=128)

            wqk_all = wpool.tile([128, NPAIR * 2 * D], FP8, name="wqk_all")
            wqk_t = {
                p: wqk_all[:, p * 2 * D : (p + 1) * 2 * D] for p in range(NPAIR)
            }
            wq_sb = {p: wqk_t[p][:, 0:D] for p in range(NPAIR)}
            wk_sb = {p: wqk_t[p][:, D : 2 * D] for p in range(NPAIR)}
            x8_all = wpool.tile([128, NDC * 2 * S], FP8, name="x8_all")
            # per d-chunk: [128, 2, 1024] = (hi cols 0:1024, lo cols 1024:2048)
            x8 = [
                x8_all[:, dc * 2 * S : (dc + 1) * 2 * S].rearrange(
                    "p (two s) -> p two s", two=2
                )
                for dc in range(NDC)
            ]
            x8_src = x8_d.rearrange("(dc p) s -> p dc s", p=128)

            # two hwdge queues (SP + ACT): critical-path tensors first
            bq_sb = const.tile([128, NPAIR], F32)
            wv_all = wpool.tile([128, NDC * D], BF16, name="wv_all")
            wv_sb = [wv_all[:, dc * D : (dc + 1) * D] for dc in range(NDC)]
            bv_b = const.tile([128, D], F32)
            bo_b = const.tile([128, D], F32)
            ident = const.tile([128, 128], BF16)
            wo_all = wpool.tile([128, NDC * D], BF16, name="wo_all")
            wo_sb = [wo_all[:, dc * D : (dc + 1) * D] for dc in range(NDC)]

            nc.sync.dma_start(wqk_t[0][:, 0:D], wqk_d[0, :, 0:D])
            x8_dst = x8_all.rearrange("p (dc two s) -> p dc two s", dc=NDC, two=2)
            for dc in range(NDC):
                eng = (nc.sync, nc.scalar)[dc % 2]
                eng.dma_start(x8_dst[:, dc, :, :],
                              x8_src[:, dc, :].rearrange("p (two s) -> p two s", two=2))
                if dc == 1:
                    nc.scalar.dma_start(bq_sb, bq_d[:, :])
                if dc == 3:
                    nc.scalar.dma_start(
                        wqk_t[0][:, D : 2 * D], wqk_d[0, :, D : 2 * D]
                    )
            for dc in range(NDC):
                eng = (nc.sync, nc.scalar)[dc % 2]
                eng.dma_start(xT[dc], xt_src[:, dc, :])
            for p in range(1, NPAIR):
                nc.sync.dma_start(wqk_t[p], wqk_d[p, :, :])
            for dc in range(NDC):
                nc.sync.dma_start(wv_sb[dc], wv_d[dc, :, :])
            nc.sync.dma_start(
                bv_b, bass.AP(tensor=bv_d, offset=0, ap=[[0, 128], [1, D]])
            )
            nc.sync.dma_start(
                bo_b, bass.AP(tensor=bo_d, offset=0, ap=[[0, 128], [1, D]])
            )
            nc.sync.dma_start(ident, ident_d[:, :])
            for dc in range(NDC):
                nc.sync.dma_start(wo_sb[dc], wo_d[dc, :, :])

            # persistent V tiles: [t 128, 12*65] (col h*65+64 is the ones col)
            VW = H * 65  # 780
            v_t = [vpool.tile([128, VW], BF16, name=f"V{t}") for t in range(NTC)]
            # persistent OT tiles per pair: [of 128, 1024 s]
            OT = [ot_pool.tile([128, S], BF16, name=f"OT{p}") for p in range(NPAIR)]
            # partial y (pairs 0..4 + bias), per s-chunk, bf16
            ypart = yp_pool.tile([128, NSC * D], BF16, name="ypart")

            warm = const.tile([128, 512], BF16, name="warm")
            nc.vector.memset(warm, 0.0)
            for i in range(5):
                wps = ps.tile([128, 512], F32, tag="pp", bufs=2, name=f"warm{i}")
                nc.tensor.matmul(
                    wps[:, 0:256], warm[:, 0:128], warm[:, 0:256],
                    start=True, stop=True,
                )

            q8 = {}
            k8 = {}
            pts = {}  # pts[p][h2][tcb] -> P^T tile

            # ---------- chunk emitters (each ~0.4-1.6us of PE work) ----------

            proj_pst = {}

            def proj_chunk(p, which, nh, part=None):
                """One 512-col half of a q/k projection + its evacuation.

                part=0: first 3 d-chunks; part=1: last 3 + evac; None: all.
                """
                w = wq_sb[p] if which == "q" else wk_sb[p]
                if part == 1:
                    pst = proj_pst.pop((p, which, nh))
                else:
                    pst = ps.tile(
                        [128, 512], F32, tag="pp", bufs=2, name=f"pj{p}{which}{nh}"
                    )
                rng = {None: range(NDC), 0: range(3), 1: range(3, NDC)}[part]
                for dc in rng:
                    nc.tensor.matmul(
                        pst,
                        w[:, dc * 128 : (dc + 1) * 128]
                        .unsqueeze(1)
                        .broadcast_to((128, 2, 128)),
                        x8[dc][:, :, nh * 512 : (nh + 1) * 512],
                        start=(dc == 0),
                        stop=(dc == NDC - 1),
                        perf_mode=DR,
                    )
                if part == 0:
                    proj_pst[(p, which, nh)] = pst
                    return
                sl = slice(nh * 512, (nh + 1) * 512)
                with tc.high_priority(offset=200):
                    if which == "q":
                        hi = q8[p][:, 0:S]
                        lo = q8[p][:, S : 2 * S]
                        nc.vector.tensor_scalar_add(
                            hi[:, sl], pst, bq_sb[:, p : p + 1]
                        )
                        nc.vector.scalar_tensor_tensor(
                            lo[:, sl], pst, bq_sb[:, p : p + 1], hi[:, sl],
                            Alu.add, Alu.subtract,
                        )
                    else:
                        nc.vector.tensor_copy(k8[p][:, sl], pst)

            def score_chunk(p, h2, tcb, split_exp=False):
                """One scores tile (2 fp8-DR matmuls) + exp -> P^T tile.

                split_exp: exp per s-half right behind its matmul — used for
                the first tiles so ACT starts before the q-nh1 evac lands.
                """
                q3 = q8[p].rearrange("p (two s) -> p two s", two=2)
                psl = slice(h2 * 64, (h2 + 1) * 64)
                st = ps.tile([128, S], F32, tag="sc", bufs=2, name=f"s{p}{h2}{tcb}")
                lhsT = (
                    k8[p][psl, tcb * 128 : (tcb + 1) * 128]
                    .unsqueeze(1)
                    .broadcast_to((64, 2, 128))
                )
                pt = pt_pool.tile([128, S], BF16, tag="pt", name=f"pt{p}{h2}{tcb}")
                for sh in range(2):
                    ssl = slice(sh * 512, (sh + 1) * 512)
                    nc.tensor.matmul(
                        st[:, ssl],
                        lhsT,
                        q3[psl, :, ssl],
                        start=True,
                        stop=True,
                        perf_mode=DR,
                    )
                    if split_exp:
                        nc.scalar.activation(pt[:, ssl], st[:, ssl], Exp, scale=0.125)
                if not split_exp:
                    nc.scalar.activation(pt, st, Exp, scale=0.125)
                pts[p][h2][tcb] = pt

            def v_chunk(tcb, half):
                """v-proj for one (t-chunk, col-half); swapped operands."""
                c0, c1 = ((0, 512), (512, 768))[half]
                n = c1 - c0
                pv = ps.tile([128, 512], F32, tag="pp", bufs=2, name=f"vv{tcb}{half}")
                for dc in range(NDC):
                    nc.tensor.matmul(
                        pv[:, 0:n],
                        xT[dc][:, tcb * 128 : (tcb + 1) * 128],
                        wv_sb[dc][:, c0:c1],
                        start=(dc == 0),
                        stop=(dc == NDC - 1),
                    )
                nh = n // 64
                h0 = c0 // 64
                dst = v_t[tcb][:, h0 * 65 : h0 * 65 + nh * 65]
                dst3 = dst.rearrange("p (h e) -> p h e", e=65)[:, :, 0:64]
                src3 = pv[:, 0:n].rearrange("p (h e) -> p h e", e=64)
                bias3 = bv_b[:, c0:c1].rearrange("p (h e) -> p h e", e=64)
                nc.vector.tensor_tensor(dst3, src3, bias3, Alu.add)
                if half == 1:
                    ones3 = v_t[tcb].rearrange("p (h e) -> p h e", e=65)[:, :, 64:65]
                    nc.vector.memset(ones3, 1.0)

            onrm_t = {}

            av_ov = {}

            def av_chunk(p, sc, heads=(0, 1), transpose="dma", quarter=None):
                """O for one (pair, s-chunk): AV matmuls, normalize, transpose.

                transpose: "dma" (SP hwdge queue), "pe" (tail: PE + Pool copy),
                or None (first-head half of a split pair).
                """
                if (p, sc) in onrm_t:
                    onrm = onrm_t.pop((p, sc))
                else:
                    onrm = on_pool.tile(
                        [128, 128], BF16, tag="on", name=f"on{p}{sc}"
                    )
                if quarter is not None:
                    heads = (quarter // 2,)
                    tcbs = range(4) if quarter % 2 == 0 else range(4, NTC)
                else:
                    tcbs = range(NTC)
                for h2 in heads:
                    h = 2 * p + h2
                    if quarter is not None and quarter % 2 == 1:
                        ov = av_ov.pop((p, sc, h2))
                    else:
                        ov = ps.tile(
                            [128, 65], F32, tag="ov", bufs=2, name=f"ov{p}{sc}{h2}"
                        )
                    for tcb in tcbs:
                        nc.tensor.matmul(
                            ov,
                            pts[p][h2][tcb][:, sc * 128 : (sc + 1) * 128],
                            v_t[tcb][:, h * 65 : h * 65 + 65],
                            start=(tcb == 0),
                            stop=(tcb == NTC - 1),
                        )
                    if quarter is not None and quarter % 2 == 0:
                        av_ov[(p, sc, h2)] = ov
                        onrm_t[(p, sc)] = onrm
                        return
                    rsb = ov_sb_pool.tile(
                        [128, 1], F32, tag="ovsb", name=f"r{p}{sc}{h2}"
                    )
                    import contextlib as _cl
                    hp = tc.high_priority(offset=250) if p == 5 else _cl.nullcontext()
                    with hp:
                        nc.vector.reciprocal(out=rsb, in_=ov[:, 64:65])
                        nc.vector.tensor_scalar_mul(
                            onrm[:, h2 * 64 : (h2 + 1) * 64], ov[:, 0:64], rsb
                        )
                if quarter is not None and quarter < 3:
                    onrm_t[(p, sc)] = onrm
                    return
                if transpose is None:
                    onrm_t[(p, sc)] = onrm
                elif transpose == "dma":
                    nc.sync.dma_start_transpose(
                        OT[p][:, sc * 128 : (sc + 1) * 128], onrm
                    )
                else:
                    tp = ps.tile([128, 128], BF16, tag="ov", bufs=2, name=f"tp{sc}")
                    nc.tensor.transpose(tp, onrm, ident)
                    with tc.high_priority(offset=250):
                        nc.vector.tensor_copy(
                            OT[p][:, sc * 128 : (sc + 1) * 128], tp
                        )

            def ypart_chunk(sc, half=None):
                """Out-proj partial: pairs 0..4 + bias -> bf16 ypart."""
                cols = ((0, 512), (512, 768))
                if half is not None:
                    cols = (cols[half],)
                for c0, c1 in cols:
                    n = c1 - c0
                    yp = ps.tile([128, 512], F32, tag="pp", bufs=2, name=f"yp{sc}")
                    for p in range(NPAIR - 1):
                        nc.tensor.matmul(
                            yp[:, 0:n],
                            OT[p][:, sc * 128 : (sc + 1) * 128],
                            wo_sb[p][:, c0:c1],
                            start=(p == 0),
                            stop=(p == NPAIR - 2),
                        )
                    nc.vector.tensor_add(
                        ypart[:, sc * D + c0 : sc * D + c1], yp[:, 0:n],
                        bo_b[:, c0:c1],
                    )

            def yfinal_chunk(sc):
                """Fold pair 5 + the bf16 partial (identity matmul), DMA out
                straight from PSUM."""
                yf = ps.tile([128, D], F32, tag="sc", bufs=2, name=f"yf{sc}")
                lhsT = OT[NPAIR - 1][:, sc * 128 : (sc + 1) * 128]
                for c0, c1 in ((0, 512), (512, 768)):
                    nc.tensor.matmul(
                        yf[:, c0:c1], lhsT, wo_sb[NPAIR - 1][:, c0:c1],
                        start=True, stop=(sc % 2 == 0),
                    )
                    if sc % 2 == 1:
                        # ACT does the evac copy; fold the partial on the PE
                        nc.tensor.matmul(
                            yf[:, c0:c1], ident,
                            ypart[:, sc * D + c0 : sc * D + c1],
                            start=False, stop=True,
                        )
                ysb = y_pool.tile([128, D], F32, tag="y", name=f"y{sc}")
                if sc % 2 == 0:
                    # DVE folds the partial during the evac (same cost as copy)
                    with tc.high_priority(offset=250):
                        nc.vector.tensor_add(
                            ysb, yf, ypart[:, sc * D : (sc + 1) * D]
                        )
                else:
                    nc.scalar.copy(ysb, yf)
                eng = (nc.sync, nc.scalar)[sc % 2]
                eng.dma_start(y_d[sc * 128 : (sc + 1) * 128, :], ysb)

            # ---------- interleaved schedule ----------
            # Filler work (est PE ns, thunk) emitted between score chunks so
            # the in-order PE stream never blocks on the scores/exp ping-pong.

            def alloc_pair(p):
                q8[p] = qk8.tile([128, 2 * S], FP8, tag="q8", name=f"q8_{p}")
                k8[p] = qk8.tile([128, S], FP8, tag="k8", name=f"k8_{p}")
                pts[p] = [[None] * NTC for _ in range(2)]

            def proj_fillers(p):
                def mk(which, nh, part):
                    return lambda: proj_chunk(p, which, nh, part)

                out = []
                for which, nh in (("q", 0), ("k", 0), ("q", 1), ("k", 1)):
                    out.append((340, mk(which, nh, 0)))
                    out.append((340, mk(which, nh, 1)))
                return out

            def av_fillers(p):
                return [
                    (480, (lambda sc_: lambda: av_chunk(p, sc_))(sc))
                    for sc in range(NSC)
                ]

            def av_fillers_q(p, interleave=None, transpose="dma"):
                out = []
                for sc in range(NSC):
                    for q in range(4):
                        out.append(
                            (150,
                             (lambda sc_, q_: lambda: av_chunk(
                                 p, sc_, quarter=q_, transpose=transpose))(sc, q))
                        )
                    if interleave is not None and sc >= 3:
                        out.extend(interleave(sc - 3))
                if interleave is not None:
                    for sc in (NSC - 3, NSC - 2, NSC - 1):
                        out.extend(interleave(sc))
                return out

            v_fillers = [
                (1350 if half == 0 else 700,
                 (lambda t_, h_: lambda: v_chunk(t_, h_))(tcb, half))
                for tcb in range(NTC)
                for half in range(2)
            ]

            def av5_h0_fillers():
                # head-10 AV split into half-chunks (4 mms each) so blocked
                # instrs fit the 4-deep PE wait queue.
                out = []
                for sc in range(NSC):
                    for q in (0, 1):
                        out.append(
                            (130,
                             (lambda sc_, q_: lambda: av_chunk(
                                 5, sc_, quarter=q_, transpose=None))(sc, q))
                        )
                return out

            def ypart_interleave(sc):
                return [
                    (1100 if h == 0 else 580,
                     (lambda sc_, h_: lambda: ypart_chunk(sc_, h_))(sc, h))
                    for h in range(2)
                ]

            fillers_by_pair = {
                0: proj_fillers(1),
                1: v_fillers[:8] + proj_fillers(2),
                2: v_fillers[8:] + proj_fillers(3) + av_fillers(0),
                3: proj_fillers(4) + av_fillers(1),
                4: proj_fillers(5) + av_fillers(2) + av_fillers_q(3),
                5: av_fillers_q(4) + av5_h0_fillers()
                   + [f for sc in range(NSC) for f in ypart_interleave(sc)],
            }

            alloc_pair(0)
            proj_chunk(0, "q", 0)
            proj_chunk(0, "k", 0)
            # first two score tiles, s-half 0 only (q-nh1 not needed yet):
            # emitted as split matmuls so ACT starts ~2us earlier.
            intro_st = {}
            q3_0 = q8[0].rearrange("p (two s) -> p two s", two=2)
            for tcb in range(2):
                st = ps.tile([128, S], F32, tag="sc", bufs=2, name=f"i{tcb}")
                pt = pt_pool.tile([128, S], BF16, tag="pt", name=f"ipt{tcb}")
                lhsT = (
                    k8[0][0:64, tcb * 128 : (tcb + 1) * 128]
                    .unsqueeze(1)
                    .broadcast_to((64, 2, 128))
                )
                nc.tensor.matmul(
                    st[:, 0:512], lhsT, q3_0[0:64, :, 0:512],
                    start=True, stop=True, perf_mode=DR,
                )
                nc.scalar.activation(pt[:, 0:512], st[:, 0:512], Exp, scale=0.125)
                intro_st[tcb] = (st, pt, lhsT)
            proj_chunk(0, "q", 1)
            for tcb in range(2):
                st, pt, lhsT = intro_st[tcb]
                nc.tensor.matmul(
                    st[:, 512:S], lhsT, q3_0[0:64, :, 512:S],
                    start=True, stop=True, perf_mode=DR,
                )
                nc.scalar.activation(pt[:, 512:S], st[:, 512:S], Exp, scale=0.125)
                pts[0][0][tcb] = pt
            fillers_by_pair[0] = [(680, lambda: proj_chunk(0, "k", 1))] + \
                fillers_by_pair[0]

            FILL_NS = 580  # target filler PE-ns per score chunk
            for p in range(NPAIR):
                if p + 1 < NPAIR:
                    alloc_pair(p + 1)
                fill = list(fillers_by_pair[p])
                fi = 0
                for h2 in range(2):
                    for tcb in range(NTC):
                        if p == 0 and h2 == 0 and tcb < 2:
                            continue  # emitted in the intro above
                        score_chunk(p, h2, tcb)
                        budget = FILL_NS
                        while fi < len(fill) and budget > 0:
                            est, thunk = fill[fi]
                            thunk()
                            budget -= est
                            fi += 1
                while fi < len(fill):
                    fill[fi][1]()
                    fi += 1
                if p >= 2:
                    del pts[p - 2]

            # tail: pair-5 head-11 AV per s-chunk (head 10 was a filler),
            # PE-transpose + Pool copy (no DMA queue latency), then the final
            # out-proj chunk for that s-chunk.
            for sc in range(NSC):
                av_chunk(5, sc, heads=(1,), transpose="pe")
                yfinal_chunk(sc)

    nc.compile()
    return nc


def _prep_inputs(x, Wq, bq, Wk, bk, Wv, bv, Wo, bo):
    """Host-side layout transforms + bf16 casts."""
    x = np.asarray(x)
    xT = np.ascontiguousarray(x.transpose(0, 2, 1)).astype(_BF16)
    xt_f32 = np.ascontiguousarray(x.transpose(0, 2, 1)).astype(np.float32)
    x8_hi = xt_f32.astype(_FP8)
    x8_lo = (xt_f32 - x8_hi.astype(np.float32)).astype(_FP8)
    # [B, D, 2, S] -> [B, D, 2*S]
    x8 = np.stack([x8_hi, x8_lo], axis=2).reshape(B, D, 2 * S)

    def pack_w(W):
        # W [H, D, DH] -> [NPAIR, 128(d_sub), D] where col dc*128+m holds
        # W[2p + m//64, dc*128+d_sub, m%64]
        Wp = np.empty((NPAIR, 128, D), np.float32)
        W = np.asarray(W, np.float32)
        for p in range(NPAIR):
            blk = np.concatenate([W[2 * p], W[2 * p + 1]], axis=1)  # [D, 128]
            Wp[p] = blk.reshape(NDC, 128, 128).transpose(1, 0, 2).reshape(128, D)
        return Wp

    wqk = np.concatenate([pack_w(Wq), pack_w(Wk)], axis=2).astype(_FP8)

    bq_sb = np.empty((128, NPAIR), np.float32)
    bq = np.asarray(bq, np.float32)
    for p in range(NPAIR):
        bq_sb[:, p] = np.concatenate([bq[2 * p], bq[2 * p + 1]])

    # Wv natural [D, H*DH] chunked over d
    Wv = np.asarray(Wv, np.float32)
    wv = Wv.transpose(1, 0, 2).reshape(D, D).reshape(NDC, 128, D).astype(_BF16)
    Wo = np.asarray(Wo, np.float32)
    wo = Wo.reshape(NDC, 128, D).astype(_BF16)

    bv_h = np.asarray(bv, np.float32).reshape(1, D)
    bo_h = np.asarray(bo, np.float32).reshape(1, D)

    shared = {
        "wqk8": wqk,
        "wv": wv,
        "wo": wo,
        "bq": bq_sb,
        "bv": bv_h,
        "bo": bo_h,
        "ident": np.eye(128, dtype=np.float32).astype(_BF16),
    }
    return xT, x8, shared


def kernel(x, Wq, bq, Wk, bk, Wv, bv, Wo, bo):
    from concourse.bass_utils import run_bass_kernel_spmd

    if "nc" not in _cache:
        _cache["nc"] = _build_program()
    nc = _cache["nc"]

    xT, x8, shared = _prep_inputs(x, Wq, bq, Wk, bk, Wv, bv, Wo, bo)
    in_maps = [
        dict(shared, xT=np.ascontiguousarray(xT[b]), x8=np.ascontiguousarray(x8[b]))
        for b in range(B)
    ]
    res = run_bass_kernel_spmd(nc, in_maps, core_ids=list(range(B)))
    y = np.stack([res.results[b]["y"] for b in range(B)], axis=0)
    return y.astype(np.float32)
